# revision 36
# baseline (speedup 1.0000x reference)
"""Trainium2 Bass kernel for nn_MixtureLayer (MoE routing, 8 experts, top-2,
grouped capacity routing + shared expert).

Strategy: data-parallel over the 128 token-groups -> 16 groups per core.
Each core runs the router, dispatch, all 8 experts' FFNs on its own groups,
the shared expert, and the combine.  No collectives needed.

Weights are pre-cast to bf16 on the host (they are consumed exactly once on
device, so an on-device cast would only add DMA + vector work).  The router
(logits/softmax/top-k/cumsum) runs entirely in fp32 so expert selection
matches the jax reference; the FFN matmuls run bf16 with fp32 PSUM.

Schedule: the 16 router groups are processed in 4 quarters, software-
pipelined with the 4 shared-expert FFN units (which depend only on x^T, not
on routing) so the PE array stays busy during the router's serial DVE
chains.  The 8 routed-expert units follow, and the combine is pipelined
directly behind the last expert's FFN2.
"""

import sys
import types

import numpy as np
import ml_dtypes

try:  # concourse is normally on sys.path via the container's site setup
    import concourse.bass as bass  # noqa: F401
except ImportError:  # pragma: no cover
    sys.path.insert(0, "/opt/trn_rl_repo")

import concourse.bass as bass
import concourse.tile as tile
from concourse import bacc, mybir
from concourse.bass_utils import run_bass_kernel_spmd

F32 = mybir.dt.float32
BF16 = mybir.dt.bfloat16
AF = mybir.ActivationFunctionType
ALU = mybir.AluOpType
GELU = AF.Gelu_apprx_tanh  # jax.nn.gelu(approximate=True)

# ---- problem constants (hardcoded from the spec) ----
NCORES = 8
D, H, E = 1024, 4096, 8
B, S = 8, 2048
GRP = 128                 # tokens per routing group
NG_TOT = 128              # total groups
NG = NG_TOT // NCORES     # groups per core = 16
TOK = NG * GRP            # tokens per core = 2048
CAP = 32                  # capacity slots per (group, expert); slot 31 unused
DC = D // 128             # 8 chunks of d
HC = H // 128             # 32 chunks of h
SLOTS = NG * CAP          # 512 slots per expert per core

_CACHE = {}


def _ensure_ntff_hook():
    """Register the axon NTFF profiling hook if the image's antenv stub lacks
    it (needed only when tracing; harmless otherwise)."""
    try:
        import antenv
    except ImportError:
        return
    if "antenv.axon_hooks" in sys.modules:
        return
    m = types.ModuleType("antenv.axon_hooks")
    m._hook = None

    def _set(h, _m=m):
        _m._hook = h

    def _get(_m=m):
        return _m._hook

    m.set_axon_ntff_profile_hook = _set
    m.get_axon_ntff_profile_hook = _get
    sys.modules["antenv.axon_hooks"] = m
    antenv.axon_hooks = m
    try:
        from trn_agent_boot.trn_boot import _ntff_profile_via_ctypes

        hook = _ntff_profile_via_ctypes("/opt/axon/libaxon_pjrt.so")
        if hook is not None:
            _set(hook)
    except Exception:
        pass


def _emit_ffn_unit(nc, pools, rhs_fn, out_ap_fn, keys_ap, vals_ap,
                   weave=None, post_ffn1=None):
    """One FFN 'unit': 512 input columns (slots/tokens) through d->h gelu h->d.

    rhs_fn(dc) -> AP [128, 512] of the input in transposed layout (d on
    partitions).  keys_ap [D, H] / vals_ap [H, D] are bf16 DRAM APs.
    weave(b) is called after FFN1 block b's gelus (PSUM has >=4 free banks
    there); post_ffn1() after the last block, before FFN2 claims all 8.
    """
    hid = [pools["hid"].tile([128, 512], BF16, tag=f"hid{hc}", name=f"hid{hc}") for hc in range(HC)]
    # FFN1: hid[hc][128, 512] = gelu(sum_dc keys[dc,hc].T @ rhs[dc])
    # The first 8 values tiles are requested one-per-block from inside FFN1
    # (exactly the vb ring depth, so the doorbells never block the gelus) —
    # otherwise FFN2 stalls several us at its start waiting for the values
    # stream to spin up.
    vb_pre = []
    for hcb in range(8):  # blocks of 4 h-chunks
        eps = [pools["ps"].tile([128, 512], F32, tag="ps", name="ps") for _ in range(4)]
        for dc in range(DC):
            kb = pools["kb"].tile([128, 512], BF16, tag="kb", name="kb")
            nc.sync.dma_start(kb[:], keys_ap[dc * 128:(dc + 1) * 128,
                                             hcb * 512:(hcb + 1) * 512])
            rhs = rhs_fn(dc)
            for hh in range(4):
                nc.tensor.matmul(eps[hh][:], kb[:, hh * 128:(hh + 1) * 128], rhs,
                                 start=(dc == 0), stop=(dc == DC - 1))
        for hh in range(4):
            nc.scalar.activation(hid[hcb * 4 + hh][:], eps[hh][:], GELU)
        vb = pools["vb"].tile([128, 1024], BF16, tag="vb", name="vb")
        nc.scalar.dma_start(vb[:], vals_ap[hcb * 128:(hcb + 1) * 128, :])
        vb_pre.append(vb)
        if weave is not None:
            weave(hcb)
    if post_ffn1 is not None:
        post_ffn1()
    # FFN2: out[sc*128.., 1024] = sum_hc hid[hc][:,sc].T @ values[hc]
    # values stream on the scalar (Activation) HWDGE queue so they are not
    # head-of-line blocked behind the keys stream on sync
    pss = [[pools["ps"].tile([128, 512], F32, tag="ps", name="ps") for _ in range(2)]
           for _ in range(4)]
    for hc in range(HC):
        if hc < 8:
            vb = vb_pre[hc]
        else:
            vb = pools["vb"].tile([128, 1024], BF16, tag="vb", name="vb")
            nc.scalar.dma_start(vb[:], vals_ap[hc * 128:(hc + 1) * 128, :])
        for sc in range(4):
            lhsT = hid[hc][:, sc * 128:(sc + 1) * 128]
            nc.tensor.matmul(pss[sc][0][:], lhsT, vb[:, 0:512],
                             start=(hc == 0), stop=(hc == HC - 1))
            nc.tensor.matmul(pss[sc][1][:], lhsT, vb[:, 512:1024],
                             start=(hc == 0), stop=(hc == HC - 1))
    # drain PSUM on both scalar and vector so the banks free up fast for
    # the next unit's FFN1 accumulators
    eo_tiles = []
    for sc in range(4):
        eo = pools["eo"].tile([128, 1024], BF16, tag="eo", name="eo")
        nc.scalar.copy(eo[:, 0:512], pss[sc][0][:])
        nc.vector.tensor_copy(eo[:, 512:1024], pss[sc][1][:])
        if out_ap_fn is not None:
            nc.gpsimd.dma_start(out_ap_fn(sc), eo[:])
        eo_tiles.append(eo)
    return eo_tiles


def _build_program():
    nc = bacc.Bacc("TRN2", target_bir_lowering=False, debug=False,
                   num_devices=NCORES)

    x_d = nc.dram_tensor("x_s", [TOK, D], F32, kind="ExternalInput").ap()
    gw_d = nc.dram_tensor("gw", [D, E], F32, kind="ExternalInput").ap()
    gb_d = nc.dram_tensor("gb", [1, E], F32, kind="ExternalInput").ap()
    k_d = nc.dram_tensor("keys", [E, D, H], BF16, kind="ExternalInput").ap()
    v_d = nc.dram_tensor("values", [E, H, D], BF16, kind="ExternalInput").ap()
    sk_d = nc.dram_tensor("shk", [D, H], BF16, kind="ExternalInput").ap()
    sv_d = nc.dram_tensor("shv", [H, D], BF16, kind="ExternalInput").ap()
    out_d = nc.dram_tensor("out", [TOK, D], BF16, kind="ExternalOutput").ap()

    from contextlib import ExitStack
    with tile.TileContext(nc) as tc, ExitStack() as es_glob:
        es_dT, es_xtb = ExitStack(), ExitStack()
        es_ffn, es_rt, es_cb = ExitStack(), ExitStack(), ExitStack()

        def mk(es, name, bufs, space="SBUF"):
            return es.enter_context(tc.tile_pool(name=name, bufs=bufs,
                                                 space=space))

        # global pools (live for whole kernel)
        ps = mk(es_glob, "ps", 8, "PSUM")
        const = mk(es_glob, "const", 1)
        dram = mk(es_glob, "dram", 1, "DRAM")
        p_ct = mk(es_glob, "p_ct", 1)
        pools = {"ps": ps}

        # ---------- constants ----------
        ones128 = const.tile([128, 128], F32, tag="ones128", name="ones128")
        nc.vector.memset(ones128[:], 1.0)
        ident = const.tile([128, 128], F32, tag="ident", name="ident")
        nc.gpsimd.affine_select(ident[:], ones128[:], pattern=[[1, 128]],
                                base=0, channel_multiplier=-1,
                                compare_op=ALU.is_equal, fill=0.0)
        utri = const.tile([128, 128], F32, tag="utri", name="utri")
        nc.gpsimd.affine_select(utri[:], ones128[:], pattern=[[1, 128]],
                                base=0, channel_multiplier=-1,
                                compare_op=ALU.is_ge, fill=0.0)
        # iota over capacity slots: value c+1 at slot c (c<31), -1 at c=31
        iota_f = const.tile([128, E * CAP], F32, tag="iota_f", name="iota_f")
        nc.gpsimd.iota(iota_f[:], pattern=[[0, E], [1, CAP]], base=1,
                       channel_multiplier=0,
                       allow_small_or_imprecise_dtypes=True)
        iota_3d = iota_f[:].rearrange("p (e c) -> p e c", e=E)
        nc.vector.memset(iota_3d[:, :, CAP - 1:CAP], -1.0)
        # gw/gb DMAs are issued inside emit_A(0), ordered around the first
        # x-group loads on sync, so neither the first transposes nor the
        # first logits matmuls wait on a cold queue
        gw_sb = const.tile([128, DC * E], F32, tag="gw", name="gw")
        gb_sb = const.tile([1, E], F32, tag="gb", name="gb")
        ones1 = const.tile([1, 128], F32, tag="ones1", name="ones1")
        nc.vector.memset(ones1[:], 1.0)
        gbb = const.tile([128, E], F32, tag="gbb", name="gbb")

        # ---------- persistent tensors ----------
        p_dT = mk(es_dT, "p_dT", 1)
        # xTb lives per-quarter (512 tokens) in a depth-2 ring: shared FFN
        # unit q consumes quarter q shortly after the router writes it
        p_xtb = mk(es_xtb, "p_xtb", 2)
        xTb_q = [None] * 4  # quarter -> [DC] tiles of [128, 512] bf16
        combT = [p_ct.tile([128, NG * 128], BF16, tag=f"ct{ch}",
                           name=f"ct{ch}") for ch in range(2)]
        dispT = [p_dT.tile([128, NG * E * CAP], BF16, tag=f"dT{dc}",
                           name=f"dT{dc}") for dc in range(DC)]

        # DRAM scratch
        eo_dram = [dram.tile([NG * 128, D], BF16, tag=f"eo_dram{h}",
                             name=f"eo_dram{h}") for h in range(2)]
        sh_dram = dram.tile([TOK, D], BF16, tag="sh_dram", name="sh_dram")

        # ---------- FFN pools (open early: shared units interleave) ----------
        pools["kb"] = mk(es_ffn, "p_kb", 10)
        pools["vb"] = mk(es_ffn, "p_vb", 8)
        pools["hid"] = mk(es_ffn, "p_hid", 1)
        pools["eo"] = mk(es_ffn, "p_eo", 4)

        # ---------- router pools ----------
        p_xg = mk(es_rt, "p_xg", 2)
        p_xgb = mk(es_rt, "p_xgb", 7)
        p_xtf = mk(es_rt, "p_xtf", 6)
        p_sm = mk(es_rt, "p_sm", 24)
        p_sm8 = mk(es_rt, "p_sm8", 32)
        p_cmp = mk(es_rt, "p_cmp", 3)
        p_cb = mk(es_rt, "p_cb", 6)
        p_dm = mk(es_rt, "p_dm", 6)

        # per-group state carried between pipeline stages
        st_xgb = [None] * NG
        st_xtf = [None] * NG   # [2] tiles of [128,512] f32
        st_lg = [None] * NG    # logits in SBUF f32 [128, E]
        st_mask = [None] * NG  # (mask1, mask2)
        st_m = [None] * NG     # (m1, m2)
        st_dm = [None] * NG    # dispatch mask bf16 [128, E*CAP]
        st_comb = [None] * NG  # combine weights f32 [128, E*CAP]

        def emit_A(q):
            """Per-group: x DMA, bf16 cast, transposes, xTb/xtf, logits."""
            gs = range(4 * q, 4 * q + 4)
            xTb_q[q] = [p_xtb.tile([128, 512], BF16, tag=f"xtb{dc}",
                                   name=f"xtb{dc}") for dc in range(DC)]
            xTb = [t[:] for t in xTb_q[q]]
            for g in gs:
                xg = p_xg.tile([128, D], F32, tag="xg", name="xg")
                for piece in range(4):
                    nc.sync.dma_start(
                        xg[:, piece * 256:(piece + 1) * 256],
                        x_d[g * 128:(g + 1) * 128,
                            piece * 256:(piece + 1) * 256])
                if q == 0 and g == 0:
                    # router-weight loads right behind the first x group
                    nc.sync.dma_start(gb_sb[:], gb_d[:])
                    for dc in range(DC):
                        nc.sync.dma_start(gw_sb[:, dc * E:(dc + 1) * E],
                                          gw_d[dc * 128:(dc + 1) * 128, :])
                    gbp = ps.tile([128, E], F32, tag="ps", name="ps")
                    nc.tensor.matmul(gbp[:], ones1[:], gb_sb[:],
                                     start=True, stop=True)
                    nc.vector.tensor_copy(gbb[:], gbp[:])
                xgb = p_xgb.tile([128, D], BF16, tag="xgb", name="xgb")
                nc.scalar.copy(xgb[:], xg[:])
                st_xgb[g] = xgb
                xtf = []
                for dc4 in range(2):
                    tp = ps.tile([128, 512], F32, tag="ps", name="ps")
                    for j in range(4):
                        dc = dc4 * 4 + j
                        nc.tensor.transpose(tp[:, j * 128:(j + 1) * 128],
                                            xg[:, dc * 128:(dc + 1) * 128],
                                            ident[:])
                    t = p_xtf.tile([128, 512], F32, tag="xtf", name="xtf")
                    nc.vector.tensor_copy(t[:], tp[:])
                    xtf.append(t)
                    gl = g - 4 * q
                    for j in range(4):
                        dc = dc4 * 4 + j
                        nc.scalar.copy(xTb[dc][:, gl * 128:(gl + 1) * 128],
                                       tp[:, j * 128:(j + 1) * 128])
                st_xtf[g] = xtf
            # logits for the quarter into one PSUM tile, then to SBUF
            lgp = ps.tile([128, 4 * E], F32, tag="ps", name="ps")
            for j, g in enumerate(gs):
                logits = lgp[:, j * E:(j + 1) * E]
                xtf = st_xtf[g]
                for dc in range(DC):
                    nc.tensor.matmul(logits,
                                     xtf[dc // 4][:, (dc % 4) * 128:
                                                  (dc % 4 + 1) * 128],
                                     gw_sb[:, dc * E:(dc + 1) * E],
                                     start=(dc == 0), stop=(dc == DC - 1))
                lg = p_sm8.tile([128, E], F32, tag="sm8", name="lg")
                nc.vector.tensor_tensor(lg[:], logits, gbb[:], op=ALU.add)
                st_lg[g] = lg

        def emit_B(q):
            """Softmax + top-2 masks (DVE/ACT only)."""
            for g in range(4 * q, 4 * q + 4):
                lg = st_lg[g]
                negm = p_sm.tile([128, 1], F32, tag="sm", name="negm")
                nc.vector.tensor_reduce(negm[:], lg[:],
                                        axis=mybir.AxisListType.X,
                                        op=ALU.max, negate=True)
                ex = p_sm8.tile([128, E], F32, tag="sm8", name="ex")
                den = p_sm.tile([128, 1], F32, tag="sm", name="den")
                nc.scalar.activation(ex[:], lg[:], AF.Exp, bias=negm[:],
                                     scale=1.0, accum_out=den[:])
                rec = p_sm.tile([128, 1], F32, tag="sm", name="rec")
                nc.vector.reciprocal(rec[:], den[:])
                probs = p_sm8.tile([128, E], F32, tag="sm8", name="probs")
                nc.vector.tensor_scalar_mul(probs[:], ex[:], rec[:])
                m1 = p_sm.tile([128, 1], F32, tag="sm", name="m1")
                nc.vector.reduce_max(m1[:], probs[:],
                                     axis=mybir.AxisListType.X)
                mask1 = p_sm8.tile([128, E], F32, tag="sm8", name="mask1")
                nc.vector.tensor_scalar(mask1[:], probs[:], m1[:], None,
                                        op0=ALU.is_ge)
                probs2 = p_sm8.tile([128, E], F32, tag="sm8", name="probs2")
                nc.vector.scalar_tensor_tensor(probs2[:], mask1[:], -1e30,
                                               probs[:], ALU.mult, ALU.add)
                m2 = p_sm.tile([128, 1], F32, tag="sm", name="m2")
                nc.vector.reduce_max(m2[:], probs2[:],
                                     axis=mybir.AxisListType.X)
                mask2 = p_sm8.tile([128, E], F32, tag="sm8", name="mask2")
                nc.vector.tensor_scalar(mask2[:], probs2[:], m2[:], None,
                                        op0=ALU.is_ge)
                st_mask[g] = (mask1, mask2)
                st_m[g] = (m1, m2)

        def emit_C(q):
            """Position cumsums (PE) + capacity compare chain (DVE)."""
            gs = range(4 * q, 4 * q + 4)
            ppq = ps.tile([128, 4 * 2 * E], F32, tag="ps", name="ps")
            for j, g in enumerate(gs):
                for ki in range(2):
                    pp = ppq[:, (j * 2 + ki) * E:(j * 2 + ki + 1) * E]
                    nc.tensor.matmul(pp, utri[:], st_mask[g][ki][:],
                                     start=True, stop=True)
            for j, g in enumerate(gs):
                pos = []
                for ki in range(2):
                    pp = ppq[:, (j * 2 + ki) * E:(j * 2 + ki + 1) * E]
                    pm = p_sm8.tile([128, E], F32, tag="sm8", name="pos")
                    nc.vector.tensor_mul(pm[:], pp, st_mask[g][ki][:])
                    pos.append(pm)
                m1, m2 = st_m[g]
                cmp1 = p_cmp.tile([128, E * CAP], F32, tag="cmp", name="cmp1")
                nc.vector.tensor_tensor(
                    cmp1[:].rearrange("p (e c) -> p e c", e=E),
                    pos[0][:].unsqueeze(2).broadcast_to([128, E, CAP]),
                    iota_3d, op=ALU.is_equal)
                cmp2 = p_cmp.tile([128, E * CAP], F32, tag="cmp", name="cmp2")
                nc.vector.tensor_tensor(
                    cmp2[:].rearrange("p (e c) -> p e c", e=E),
                    pos[1][:].unsqueeze(2).broadcast_to([128, E, CAP]),
                    iota_3d, op=ALU.is_equal)
                dm = p_dm.tile([128, E * CAP], BF16, tag="dm", name="dm")
                nc.vector.tensor_add(dm[:], cmp1[:], cmp2[:])
                st_dm[g] = dm
                cmp2s = p_cmp.tile([128, E * CAP], F32, tag="cmp",
                                   name="cmp2s")
                nc.vector.tensor_scalar_mul(cmp2s[:], cmp2[:], m2[:])
                comb = p_cb.tile([128, E * CAP], F32, tag="cb", name="comb")
                nc.vector.scalar_tensor_tensor(comb[:], cmp1[:], m1[:],
                                               cmp2s[:], ALU.mult, ALU.add)
                st_comb[g] = comb

        def emit_EF(q):
            """Dispatch matmuls + combT transposes (PE) and copies out."""
            for g in range(4 * q, 4 * q + 4):
                xgb, dm, comb = st_xgb[g], st_dm[g], st_comb[g]
                for dcp in range(4):
                    dps = ps.tile([128, 512], F32, tag="ps", name="ps")
                    for j in range(2):
                        dc = dcp * 2 + j
                        nc.tensor.matmul(dps[:, j * 256:(j + 1) * 256],
                                         xgb[:, dc * 128:(dc + 1) * 128],
                                         dm[:], start=True, stop=True)
                    # drain across three engines so the PSUM ring keeps up
                    # with the dispatch matmuls
                    for j in range(2):
                        dc = dcp * 2 + j
                        dst = dispT[dc][:, g * E * CAP:(g + 1) * E * CAP]
                        eng = (dcp * 2 + j) % 3
                        if eng == 0:
                            nc.vector.tensor_copy(dst, dps[:, j * 256:
                                                           (j + 1) * 256])
                        elif eng == 1:
                            nc.scalar.copy(dst, dps[:, j * 256:(j + 1) * 256])
                        else:
                            nc.gpsimd.tensor_copy(dst, dps[:, j * 256:
                                                            (j + 1) * 256])
                ctp = ps.tile([128, 256], F32, tag="ps", name="ps")
                for ch in range(2):
                    nc.tensor.transpose(ctp[:, ch * 128:(ch + 1) * 128],
                                        comb[:, ch * 128:(ch + 1) * 128],
                                        ident[:])
                for ch in range(2):
                    nc.vector.tensor_copy(combT[ch][:, g * 128:(g + 1) * 128],
                                          ctp[:, ch * 128:(ch + 1) * 128])

        def emit_shared(q, **kw):
            # q3's shared outputs are consumed straight from SBUF by the
            # combine; the other quarters roundtrip through sh_dram
            oaf = None
            if q < 3:
                def oaf(sc, q=q):
                    return sh_dram[q * 512 + sc * 128:
                                   q * 512 + (sc + 1) * 128, :]
            return _emit_ffn_unit(
                nc, pools,
                rhs_fn=lambda dc, q=q: xTb_q[q][dc][:],
                out_ap_fn=oaf, keys_ap=sk_d, vals_ap=sv_d, **kw)

        # ---------- software-pipelined router + shared FFN ----------
        emit_A(0); emit_B(0); emit_C(0)
        emit_A(1); emit_B(1)
        emit_shared(0)
        emit_EF(0); emit_C(1)
        emit_A(2); emit_B(2)
        emit_shared(1)
        emit_EF(1); emit_C(2)
        emit_A(3); emit_B(3)
        emit_shared(2)
        emit_EF(2); emit_C(3); emit_EF(3)
        es_rt.close()

        # ---------- combine pools (open in freed router space; LIFO: close
        # before the ffn pools) ----------
        p_eg = mk(es_cb, "p_eg", 12)
        p_shg = mk(es_cb, "p_shg", 8)
        p_ot = mk(es_cb, "p_ot", 3)
        cb_state = [None] * NG

        def cb_prefetch(g):
            eg = [p_eg.tile([128, D], BF16, tag="eg", name="eg")
                  for _ in range(2)]
            for ch in range(2):
                nc.sync.dma_start(eg[ch][:],
                                  eo_dram[ch][g * 128:(g + 1) * 128, :])
            if g < 12:
                shg = p_shg.tile([128, D], BF16, tag="shg", name="shg")
                nc.scalar.dma_start(shg[:],
                                    sh_dram[g * 128:(g + 1) * 128, :])
            else:
                shg = None  # groups 12-15 add SH3's eo tiles straight from SBUF
            cb_state[g] = (eg, shg)

        def cb_compute(g, shg_override=None):
            eg, shg = cb_state[g]
            if shg_override is not None:
                shg = shg_override
            psA = ps.tile([128, 512], F32, tag="ps", name="ps")
            psB = ps.tile([128, 512], F32, tag="ps", name="ps")
            for ch in range(2):
                lhsT = combT[ch][:, g * 128:(g + 1) * 128]
                nc.tensor.matmul(psA[:], lhsT, eg[ch][:, 0:512],
                                 start=(ch == 0), stop=(ch == 1))
                nc.tensor.matmul(psB[:], lhsT, eg[ch][:, 512:1024],
                                 start=(ch == 0), stop=(ch == 1))
            ot = p_ot.tile([128, D], BF16, tag="ot", name="ot")
            nc.vector.tensor_tensor(ot[:, 0:512], psA[:], shg[:, 0:512],
                                    op=ALU.add)
            nc.vector.tensor_tensor(ot[:, 512:1024], psB[:],
                                    shg[:, 512:1024], op=ALU.add)
            nc.gpsimd.dma_start(out_d[g * 128:(g + 1) * 128, :], ot[:])

        # ---------- routed expert units ----------
        eo_g = [t[:].rearrange("(g ec) d -> g ec d", ec=128)
                for t in eo_dram]
        for e in range(E):
            def rhs_fn(dc, e=e):
                r = dispT[dc][:].rearrange("p (g ec) -> p g ec", g=NG)
                return r[:, :, e * CAP:(e + 1) * CAP]
            def out_ap_fn(sc, e=e):
                # FFN2 tile rows are slots (g-major): row r -> group
                # sc*4 + r//32, capacity slot r%32 of expert e
                return eo_g[e // 4][sc * 4:(sc + 1) * 4,
                                    (e % 4) * CAP:(e % 4 + 1) * CAP, :]
            _emit_ffn_unit(nc, pools, rhs_fn=rhs_fn, out_ap_fn=out_ap_fn,
                           keys_ap=k_d[e], vals_ap=v_d[e])

        # last shared unit runs after the experts with the combine woven in:
        # each group's expert outputs prefetch a few FFN1 blocks ahead and its
        # 4 combine matmuls fill the PSUM-free windows between FFN1 blocks.
        # No prefetch at weave(0): an eg DMA still waiting on expert 7's eo
        # writes would head-of-line block SH3's keys stream on sync.
        def sh3_weave(b):
            if b == 0:
                return
            if b >= 2:
                cb_compute(b - 2)
            pf = {1: (0, 1), 2: (2, 3), 3: (4, 5), 4: (6, 7),
                  5: (8,), 6: (9,), 7: (10,)}[b]
            for g in pf:
                cb_prefetch(g)

        def sh3_post_ffn1():
            for g in range(6, 10):
                cb_compute(g)
            for g in range(11, 16):
                cb_prefetch(g)

        sh3_eo = emit_shared(3, weave=sh3_weave, post_ffn1=sh3_post_ffn1)
        for g in range(10, 12):
            cb_compute(g)
        for g in range(12, NG):
            cb_compute(g, shg_override=sh3_eo[g - 12])

        es_cb.close()
        es_ffn.close()
        es_xtb.close()
        es_dT.close()

    nc.compile()
    return nc


LAST_RESULT = None


def kernel(x, gate_weight, gate_bias, keys, values, shared_keys,
           shared_values, **_ignored):
    global LAST_RESULT
    _ensure_ntff_hook()
    BF = ml_dtypes.bfloat16
    x = np.ascontiguousarray(np.asarray(x, dtype=np.float32))
    gate_weight = np.ascontiguousarray(np.asarray(gate_weight, np.float32))
    gate_bias = np.ascontiguousarray(
        np.asarray(gate_bias, np.float32).reshape(1, E))
    keys = np.ascontiguousarray(np.asarray(keys, np.float32).astype(BF))
    values = np.ascontiguousarray(np.asarray(values, np.float32).astype(BF))
    shared_keys = np.ascontiguousarray(
        np.asarray(shared_keys, np.float32).reshape(D, H).astype(BF))
    shared_values = np.ascontiguousarray(
        np.asarray(shared_values, np.float32).reshape(H, D).astype(BF))

    if "nc" not in _CACHE:
        _CACHE["nc"] = _build_program()
    nc = _CACHE["nc"]

    xt = x.reshape(NCORES, TOK, D)
    in_maps = []
    for i in range(NCORES):
        in_maps.append({
            "x_s": np.ascontiguousarray(xt[i]),
            "gw": gate_weight,
            "gb": gate_bias,
            "keys": keys,
            "values": values,
            "shk": shared_keys,
            "shv": shared_values,
        })
    res = run_bass_kernel_spmd(nc, in_maps, core_ids=list(range(NCORES)))
    LAST_RESULT = res
    out = np.concatenate([res.results[i]["out"] for i in range(NCORES)],
                         axis=0)
    return out.reshape(B, S, D).astype(np.float32)


# revision 44
# speedup vs baseline: 1.0088x; 1.0088x over previous
"""Trainium2 Bass kernel for nn_MixtureLayer (MoE routing, 8 experts, top-2,
grouped capacity routing + shared expert).

Strategy: data-parallel over the 128 token-groups -> 16 groups per core.
Each core runs the router, dispatch, all 8 experts' FFNs on its own groups,
the shared expert, and the combine.  No collectives needed.

Weights are pre-cast to bf16 on the host (they are consumed exactly once on
device, so an on-device cast would only add DMA + vector work).  The router
(logits/softmax/top-k/cumsum) runs entirely in fp32 so expert selection
matches the jax reference; the FFN matmuls run bf16 with fp32 PSUM.

Schedule: the 16 router groups are processed in 4 quarters, software-
pipelined with the 4 shared-expert FFN units (which depend only on x^T, not
on routing) so the PE array stays busy during the router's serial DVE
chains.  The 8 routed-expert units follow, and the combine is pipelined
directly behind the last expert's FFN2.
"""

import sys
import types

import numpy as np
import ml_dtypes

try:  # concourse is normally on sys.path via the container's site setup
    import concourse.bass as bass  # noqa: F401
except ImportError:  # pragma: no cover
    sys.path.insert(0, "/opt/trn_rl_repo")

import concourse.bass as bass
import concourse.tile as tile
from concourse import bacc, mybir
from concourse.bass_utils import run_bass_kernel_spmd

F32 = mybir.dt.float32
BF16 = mybir.dt.bfloat16
AF = mybir.ActivationFunctionType
ALU = mybir.AluOpType
GELU = AF.Gelu_apprx_tanh  # jax.nn.gelu(approximate=True)

# ---- problem constants (hardcoded from the spec) ----
NCORES = 8
D, H, E = 1024, 4096, 8
B, S = 8, 2048
GRP = 128                 # tokens per routing group
NG_TOT = 128              # total groups
NG = NG_TOT // NCORES     # groups per core = 16
TOK = NG * GRP            # tokens per core = 2048
CAP = 32                  # capacity slots per (group, expert); slot 31 unused
DC = D // 128             # 8 chunks of d
HC = H // 128             # 32 chunks of h
SLOTS = NG * CAP          # 512 slots per expert per core

_CACHE = {}


def _ensure_ntff_hook():
    """Register the axon NTFF profiling hook if the image's antenv stub lacks
    it (needed only when tracing; harmless otherwise)."""
    try:
        import antenv
    except ImportError:
        return
    if "antenv.axon_hooks" in sys.modules:
        return
    m = types.ModuleType("antenv.axon_hooks")
    m._hook = None

    def _set(h, _m=m):
        _m._hook = h

    def _get(_m=m):
        return _m._hook

    m.set_axon_ntff_profile_hook = _set
    m.get_axon_ntff_profile_hook = _get
    sys.modules["antenv.axon_hooks"] = m
    antenv.axon_hooks = m
    try:
        from trn_agent_boot.trn_boot import _ntff_profile_via_ctypes

        hook = _ntff_profile_via_ctypes("/opt/axon/libaxon_pjrt.so")
        if hook is not None:
            _set(hook)
    except Exception:
        pass


def _emit_ffn_unit(nc, pools, rhs_fn, out_ap_fn, keys_ap, vals_ap,
                   weave=None, post_ffn1=None, kb_pre=None,
                   next_keys_ap=None):
    """One FFN 'unit': 512 input columns (slots/tokens) through d->h gelu h->d.

    rhs_fn(dc) -> AP [128, 512] of the input in transposed layout (d on
    partitions).  keys_ap [D, H] / vals_ap [H, D] are bf16 DRAM APs.
    weave(b) is called after FFN1 block b's gelus (PSUM has >=4 free banks
    there); post_ffn1() after the last block, before FFN2 claims all 8.
    kb_pre: hcb0's 8 keys tiles preloaded by the previous unit.  If
    next_keys_ap is given, this unit preloads the next unit's hcb0 keys
    during its FFN2 and returns them — otherwise each unit's FFN1 start
    stalls ~5us on the keys stream spinning up (the scheduler does not
    hoist a unit's first kb DMAs across the previous unit's FFN2).
    """
    hid = [pools["hid"].tile([128, 512], BF16, tag=f"hid{hc}", name=f"hid{hc}") for hc in range(HC)]
    # FFN1: hid[hc][128, 512] = gelu(sum_dc keys[dc,hc].T @ rhs[dc])
    # The first 8 values tiles are requested one-per-block from inside FFN1
    # (exactly the vb ring depth, so the doorbells never block the gelus) —
    # otherwise FFN2 stalls several us at its start waiting for the values
    # stream to spin up.
    vb_pre = []
    for hcb in range(8):  # blocks of 4 h-chunks
        eps = [pools["ps"].tile([128, 512], F32, tag="ps", name="ps") for _ in range(4)]
        for dc in range(DC):
            if hcb == 0 and kb_pre is not None:
                kb = kb_pre[dc]
            else:
                kb = pools["kb"].tile([128, 512], BF16, tag="kb", name="kb")
                nc.sync.dma_start(kb[:], keys_ap[dc * 128:(dc + 1) * 128,
                                                 hcb * 512:(hcb + 1) * 512])
            rhs = rhs_fn(dc)
            for hh in range(4):
                nc.tensor.matmul(eps[hh][:], kb[:, hh * 128:(hh + 1) * 128], rhs,
                                 start=(dc == 0), stop=(dc == DC - 1))
        for hh in range(4):
            nc.scalar.activation(hid[hcb * 4 + hh][:], eps[hh][:], GELU)
        vb = pools["vb"].tile([128, 1024], BF16, tag="vb", name="vb")
        nc.scalar.dma_start(vb[:], vals_ap[hcb * 128:(hcb + 1) * 128, :])
        vb_pre.append(vb)
        if weave is not None:
            weave(hcb)
    if post_ffn1 is not None:
        post_ffn1()
    # FFN2: out[sc*128.., 1024] = sum_hc hid[hc][:,sc].T @ values[hc]
    # values stream on the scalar (Activation) HWDGE queue so they are not
    # head-of-line blocked behind the keys stream on sync
    pss = [[pools["ps"].tile([128, 512], F32, tag="ps", name="ps") for _ in range(2)]
           for _ in range(4)]
    kb_next = None
    for hc in range(HC):
        if hc < 8:
            vb = vb_pre[hc]
        else:
            vb = pools["vb"].tile([128, 1024], BF16, tag="vb", name="vb")
            nc.scalar.dma_start(vb[:], vals_ap[hc * 128:(hc + 1) * 128, :])
        if hc == 20 and next_keys_ap is not None:
            kb_next = []
            for dc in range(DC):
                kb = pools["kb"].tile([128, 512], BF16, tag="kb", name="kb")
                nc.sync.dma_start(kb[:],
                                  next_keys_ap[dc * 128:(dc + 1) * 128,
                                               0:512])
                kb_next.append(kb)
        for sc in range(4):
            lhsT = hid[hc][:, sc * 128:(sc + 1) * 128]
            nc.tensor.matmul(pss[sc][0][:], lhsT, vb[:, 0:512],
                             start=(hc == 0), stop=(hc == HC - 1))
            nc.tensor.matmul(pss[sc][1][:], lhsT, vb[:, 512:1024],
                             start=(hc == 0), stop=(hc == HC - 1))
    # drain PSUM on both scalar and vector so the banks free up fast for
    # the next unit's FFN1 accumulators
    eo_tiles = []
    for sc in range(4):
        eo = pools["eo"].tile([128, 1024], BF16, tag="eo", name="eo")
        nc.scalar.copy(eo[:, 0:512], pss[sc][0][:])
        nc.vector.tensor_copy(eo[:, 512:1024], pss[sc][1][:])
        if out_ap_fn is not None:
            nc.gpsimd.dma_start(out_ap_fn(sc), eo[:])
        eo_tiles.append(eo)
    return eo_tiles, kb_next


def _build_program():
    nc = bacc.Bacc("TRN2", target_bir_lowering=False, debug=False,
                   num_devices=NCORES)

    x_d = nc.dram_tensor("x_s", [TOK, D], F32, kind="ExternalInput").ap()
    gw_d = nc.dram_tensor("gw", [D, E], F32, kind="ExternalInput").ap()
    gb_d = nc.dram_tensor("gb", [1, E], F32, kind="ExternalInput").ap()
    k_d = nc.dram_tensor("keys", [E, D, H], BF16, kind="ExternalInput").ap()
    v_d = nc.dram_tensor("values", [E, H, D], BF16, kind="ExternalInput").ap()
    sk_d = nc.dram_tensor("shk", [D, H], BF16, kind="ExternalInput").ap()
    sv_d = nc.dram_tensor("shv", [H, D], BF16, kind="ExternalInput").ap()
    out_d = nc.dram_tensor("out", [TOK, D], BF16, kind="ExternalOutput").ap()

    from contextlib import ExitStack
    with tile.TileContext(nc) as tc, ExitStack() as es_glob:
        es_dT, es_xtb = ExitStack(), ExitStack()
        es_ffn, es_rt, es_cb = ExitStack(), ExitStack(), ExitStack()

        def mk(es, name, bufs, space="SBUF"):
            return es.enter_context(tc.tile_pool(name=name, bufs=bufs,
                                                 space=space))

        # global pools (live for whole kernel)
        ps = mk(es_glob, "ps", 8, "PSUM")
        const = mk(es_glob, "const", 1)
        dram = mk(es_glob, "dram", 1, "DRAM")
        p_ct = mk(es_glob, "p_ct", 1)
        pools = {"ps": ps}

        # ---------- constants ----------
        ones128 = const.tile([128, 128], F32, tag="ones128", name="ones128")
        nc.vector.memset(ones128[:], 1.0)
        ident = const.tile([128, 128], F32, tag="ident", name="ident")
        nc.gpsimd.affine_select(ident[:], ones128[:], pattern=[[1, 128]],
                                base=0, channel_multiplier=-1,
                                compare_op=ALU.is_equal, fill=0.0)
        utri = const.tile([128, 128], F32, tag="utri", name="utri")
        nc.gpsimd.affine_select(utri[:], ones128[:], pattern=[[1, 128]],
                                base=0, channel_multiplier=-1,
                                compare_op=ALU.is_ge, fill=0.0)
        # iota over capacity slots: value c+1 at slot c (c<31), -1 at c=31
        iota_f = const.tile([128, E * CAP], F32, tag="iota_f", name="iota_f")
        nc.gpsimd.iota(iota_f[:], pattern=[[0, E], [1, CAP]], base=1,
                       channel_multiplier=0,
                       allow_small_or_imprecise_dtypes=True)
        iota_3d = iota_f[:].rearrange("p (e c) -> p e c", e=E)
        nc.vector.memset(iota_3d[:, :, CAP - 1:CAP], -1.0)
        # gw/gb DMAs are issued inside emit_A(0), ordered around the first
        # x-group loads on sync, so neither the first transposes nor the
        # first logits matmuls wait on a cold queue
        gw_sb = const.tile([128, DC * E], F32, tag="gw", name="gw")
        gb_sb = const.tile([1, E], F32, tag="gb", name="gb")
        ones1 = const.tile([1, 128], F32, tag="ones1", name="ones1")
        nc.vector.memset(ones1[:], 1.0)
        gbb = const.tile([128, E], F32, tag="gbb", name="gbb")

        # ---------- persistent tensors ----------
        p_dT = mk(es_dT, "p_dT", 1)
        # xTb lives per-quarter (512 tokens) in a depth-2 ring: shared FFN
        # unit q consumes quarter q shortly after the router writes it
        p_xtb = mk(es_xtb, "p_xtb", 2)
        xTb_q = [None] * 4  # quarter -> [DC] tiles of [128, 512] bf16
        combT = [p_ct.tile([128, NG * 128], BF16, tag=f"ct{ch}",
                           name=f"ct{ch}") for ch in range(2)]
        dispT = [p_dT.tile([128, NG * E * CAP], BF16, tag=f"dT{dc}",
                           name=f"dT{dc}") for dc in range(DC)]

        # DRAM scratch
        eo_dram = [dram.tile([NG * 128, D], BF16, tag=f"eo_dram{h}",
                             name=f"eo_dram{h}") for h in range(2)]
        sh_dram = dram.tile([TOK, D], BF16, tag="sh_dram", name="sh_dram")

        # ---------- FFN pools (open early: shared units interleave) ----------
        pools["kb"] = mk(es_ffn, "p_kb", 10)
        pools["vb"] = mk(es_ffn, "p_vb", 8)
        pools["hid"] = mk(es_ffn, "p_hid", 1)
        pools["eo"] = mk(es_ffn, "p_eo", 4)

        # ---------- router pools ----------
        p_xg = mk(es_rt, "p_xg", 4)
        p_xgb = mk(es_rt, "p_xgb", 7)
        p_xtf = mk(es_rt, "p_xtf", 6)
        p_sm = mk(es_rt, "p_sm", 24)
        p_sm8 = mk(es_rt, "p_sm8", 32)
        p_cmp = mk(es_rt, "p_cmp", 3)
        p_cb = mk(es_rt, "p_cb", 6)
        p_dm = mk(es_rt, "p_dm", 6)

        # per-group state carried between pipeline stages
        st_xgb = [None] * NG
        st_xtf = [None] * NG   # [2] tiles of [128,512] f32
        st_lg = [None] * NG    # logits in SBUF f32 [128, E]
        st_mask = [None] * NG  # (mask1, mask2)
        st_m = [None] * NG     # (m1, m2)
        st_dm = [None] * NG    # dispatch mask bf16 [128, E*CAP]
        st_comb = [None] * NG  # combine weights f32 [128, E*CAP]

        def emit_A(q):
            """Per-group: x DMA, bf16 cast, transposes, xTb/xtf, logits."""
            gs = range(4 * q, 4 * q + 4)
            xTb_q[q] = [p_xtb.tile([128, 512], BF16, tag=f"xtb{dc}",
                                   name=f"xtb{dc}") for dc in range(DC)]
            xTb = [t[:] for t in xTb_q[q]]
            for g in gs:
                xh = []
                for half in range(2):
                    t = p_xg.tile([128, 512], F32, tag="xg", name="xg")
                    cols = slice(half * 512, (half + 1) * 512)
                    if g == 0:
                        for piece in range(2):
                            c0 = half * 512 + piece * 256
                            nc.sync.dma_start(
                                t[:, piece * 256:(piece + 1) * 256],
                                x_d[g * 128:(g + 1) * 128, c0:c0 + 256])
                    else:
                        nc.sync.dma_start(t[:],
                                          x_d[g * 128:(g + 1) * 128, cols])
                    xh.append(t)
                if q == 0 and g == 0:
                    # router-weight loads right behind the first x group;
                    # gw as a single multi-fragment DMA
                    nc.sync.dma_start(gb_sb[:], gb_d[:])
                    nc.sync.dma_start(
                        gw_sb[:].rearrange("p (dc e) -> p dc e", dc=DC),
                        gw_d[:].rearrange("(dc p) e -> p dc e", p=128))
                    gbp = ps.tile([128, E], F32, tag="ps", name="ps")
                    nc.tensor.matmul(gbp[:], ones1[:], gb_sb[:],
                                     start=True, stop=True)
                    nc.vector.tensor_copy(gbb[:], gbp[:])
                xgb = p_xgb.tile([128, D], BF16, tag="xgb", name="xgb")
                nc.scalar.copy(xgb[:, 0:512], xh[0][:])
                nc.scalar.copy(xgb[:, 512:1024], xh[1][:])
                st_xgb[g] = xgb
                xtf = []
                for dc4 in range(2):
                    tp = ps.tile([128, 512], F32, tag="ps", name="ps")
                    for j in range(4):
                        nc.tensor.transpose(tp[:, j * 128:(j + 1) * 128],
                                            xh[dc4][:, j * 128:(j + 1) * 128],
                                            ident[:])
                    t = p_xtf.tile([128, 512], F32, tag="xtf", name="xtf")
                    nc.vector.tensor_copy(t[:], tp[:])
                    xtf.append(t)
                    gl = g - 4 * q
                    for j in range(4):
                        dc = dc4 * 4 + j
                        nc.scalar.copy(xTb[dc][:, gl * 128:(gl + 1) * 128],
                                       tp[:, j * 128:(j + 1) * 128])
                st_xtf[g] = xtf
            # logits for the quarter into one PSUM tile, then to SBUF
            lgp = ps.tile([128, 4 * E], F32, tag="ps", name="ps")
            for j, g in enumerate(gs):
                logits = lgp[:, j * E:(j + 1) * E]
                xtf = st_xtf[g]
                for dc in range(DC):
                    nc.tensor.matmul(logits,
                                     xtf[dc // 4][:, (dc % 4) * 128:
                                                  (dc % 4 + 1) * 128],
                                     gw_sb[:, dc * E:(dc + 1) * E],
                                     start=(dc == 0), stop=(dc == DC - 1))
                lg = p_sm8.tile([128, E], F32, tag="sm8", name="lg")
                nc.vector.tensor_tensor(lg[:], logits, gbb[:], op=ALU.add)
                st_lg[g] = lg

        def emit_B(q):
            """Softmax + top-2 masks (DVE/ACT only)."""
            for g in range(4 * q, 4 * q + 4):
                lg = st_lg[g]
                negm = p_sm.tile([128, 1], F32, tag="sm", name="negm")
                nc.vector.tensor_reduce(negm[:], lg[:],
                                        axis=mybir.AxisListType.X,
                                        op=ALU.max, negate=True)
                ex = p_sm8.tile([128, E], F32, tag="sm8", name="ex")
                den = p_sm.tile([128, 1], F32, tag="sm", name="den")
                nc.scalar.activation(ex[:], lg[:], AF.Exp, bias=negm[:],
                                     scale=1.0, accum_out=den[:])
                rec = p_sm.tile([128, 1], F32, tag="sm", name="rec")
                nc.vector.reciprocal(rec[:], den[:])
                probs = p_sm8.tile([128, E], F32, tag="sm8", name="probs")
                nc.vector.tensor_scalar_mul(probs[:], ex[:], rec[:])
                m1 = p_sm.tile([128, 1], F32, tag="sm", name="m1")
                nc.vector.reduce_max(m1[:], probs[:],
                                     axis=mybir.AxisListType.X)
                mask1 = p_sm8.tile([128, E], F32, tag="sm8", name="mask1")
                nc.vector.tensor_scalar(mask1[:], probs[:], m1[:], None,
                                        op0=ALU.is_ge)
                probs2 = p_sm8.tile([128, E], F32, tag="sm8", name="probs2")
                nc.vector.scalar_tensor_tensor(probs2[:], mask1[:], -1e30,
                                               probs[:], ALU.mult, ALU.add)
                m2 = p_sm.tile([128, 1], F32, tag="sm", name="m2")
                nc.vector.reduce_max(m2[:], probs2[:],
                                     axis=mybir.AxisListType.X)
                mask2 = p_sm8.tile([128, E], F32, tag="sm8", name="mask2")
                nc.vector.tensor_scalar(mask2[:], probs2[:], m2[:], None,
                                        op0=ALU.is_ge)
                st_mask[g] = (mask1, mask2)
                st_m[g] = (m1, m2)

        def emit_C(q):
            """Position cumsums (PE) + capacity compare chain (DVE)."""
            gs = range(4 * q, 4 * q + 4)
            ppq = ps.tile([128, 4 * 2 * E], F32, tag="ps", name="ps")
            for j, g in enumerate(gs):
                for ki in range(2):
                    pp = ppq[:, (j * 2 + ki) * E:(j * 2 + ki + 1) * E]
                    nc.tensor.matmul(pp, utri[:], st_mask[g][ki][:],
                                     start=True, stop=True)
            for j, g in enumerate(gs):
                pos = []
                for ki in range(2):
                    pp = ppq[:, (j * 2 + ki) * E:(j * 2 + ki + 1) * E]
                    pm = p_sm8.tile([128, E], F32, tag="sm8", name="pos")
                    nc.vector.tensor_mul(pm[:], pp, st_mask[g][ki][:])
                    pos.append(pm)
                m1, m2 = st_m[g]
                cmp1 = p_cmp.tile([128, E * CAP], F32, tag="cmp", name="cmp1")
                nc.vector.tensor_tensor(
                    cmp1[:].rearrange("p (e c) -> p e c", e=E),
                    pos[0][:].unsqueeze(2).broadcast_to([128, E, CAP]),
                    iota_3d, op=ALU.is_equal)
                cmp2 = p_cmp.tile([128, E * CAP], F32, tag="cmp", name="cmp2")
                nc.vector.tensor_tensor(
                    cmp2[:].rearrange("p (e c) -> p e c", e=E),
                    pos[1][:].unsqueeze(2).broadcast_to([128, E, CAP]),
                    iota_3d, op=ALU.is_equal)
                dm = p_dm.tile([128, E * CAP], BF16, tag="dm", name="dm")
                nc.vector.tensor_add(dm[:], cmp1[:], cmp2[:])
                st_dm[g] = dm
                cmp2s = p_cmp.tile([128, E * CAP], F32, tag="cmp",
                                   name="cmp2s")
                nc.vector.tensor_scalar_mul(cmp2s[:], cmp2[:], m2[:])
                comb = p_cb.tile([128, E * CAP], F32, tag="cb", name="comb")
                nc.vector.scalar_tensor_tensor(comb[:], cmp1[:], m1[:],
                                               cmp2s[:], ALU.mult, ALU.add)
                st_comb[g] = comb

        def emit_EF(q):
            """Dispatch matmuls + combT transposes (PE) and copies out."""
            for g in range(4 * q, 4 * q + 4):
                xgb, dm, comb = st_xgb[g], st_dm[g], st_comb[g]
                for dcp in range(4):
                    dps = ps.tile([128, 512], F32, tag="ps", name="ps")
                    for j in range(2):
                        dc = dcp * 2 + j
                        nc.tensor.matmul(dps[:, j * 256:(j + 1) * 256],
                                         xgb[:, dc * 128:(dc + 1) * 128],
                                         dm[:], start=True, stop=True)
                    # drain on both PSUM-capable engines so the ring keeps
                    # up with the dispatch matmuls (gpsimd cannot read PSUM)
                    for j in range(2):
                        dc = dcp * 2 + j
                        dst = dispT[dc][:, g * E * CAP:(g + 1) * E * CAP]
                        if j == 0:
                            nc.vector.tensor_copy(dst, dps[:, 0:256])
                        else:
                            nc.scalar.copy(dst, dps[:, 256:512])
                ctp = ps.tile([128, 256], F32, tag="ps", name="ps")
                for ch in range(2):
                    nc.tensor.transpose(ctp[:, ch * 128:(ch + 1) * 128],
                                        comb[:, ch * 128:(ch + 1) * 128],
                                        ident[:])
                for ch in range(2):
                    nc.vector.tensor_copy(combT[ch][:, g * 128:(g + 1) * 128],
                                          ctp[:, ch * 128:(ch + 1) * 128])

        def emit_shared(q, **kw):
            # q3's shared outputs are consumed straight from SBUF by the
            # combine; the other quarters roundtrip through sh_dram
            oaf = None
            if q < 3:
                def oaf(sc, q=q):
                    return sh_dram[q * 512 + sc * 128:
                                   q * 512 + (sc + 1) * 128, :]
            return _emit_ffn_unit(
                nc, pools,
                rhs_fn=lambda dc, q=q: xTb_q[q][dc][:],
                out_ap_fn=oaf, keys_ap=sk_d, vals_ap=sv_d, **kw)

        # ---------- software-pipelined router + shared FFN ----------
        emit_A(0); emit_B(0); emit_C(0)
        emit_A(1); emit_B(1)
        _, kbn = emit_shared(0, next_keys_ap=sk_d)
        emit_EF(0); emit_C(1)
        emit_A(2); emit_B(2)
        _, kbn = emit_shared(1, kb_pre=kbn, next_keys_ap=sk_d)
        emit_EF(1); emit_C(2)
        emit_A(3); emit_B(3)
        _, kbn = emit_shared(2, kb_pre=kbn, next_keys_ap=k_d[0])
        emit_EF(2); emit_C(3); emit_EF(3)
        es_rt.close()

        # ---------- combine pools (open in freed router space; LIFO: close
        # before the ffn pools) ----------
        p_eg = mk(es_cb, "p_eg", 12)
        p_shg = mk(es_cb, "p_shg", 8)
        p_ot = mk(es_cb, "p_ot", 3)
        cb_state = [None] * NG

        def cb_prefetch(g):
            eg = [p_eg.tile([128, D], BF16, tag="eg", name="eg")
                  for _ in range(2)]
            for ch in range(2):
                nc.sync.dma_start(eg[ch][:],
                                  eo_dram[ch][g * 128:(g + 1) * 128, :])
            if g < 12:
                shg = p_shg.tile([128, D], BF16, tag="shg", name="shg")
                nc.scalar.dma_start(shg[:],
                                    sh_dram[g * 128:(g + 1) * 128, :])
            else:
                shg = None  # groups 12-15 add SH3's eo tiles straight from SBUF
            cb_state[g] = (eg, shg)

        def cb_compute(g, shg_override=None):
            eg, shg = cb_state[g]
            if shg_override is not None:
                shg = shg_override
            psA = ps.tile([128, 512], F32, tag="ps", name="ps")
            psB = ps.tile([128, 512], F32, tag="ps", name="ps")
            for ch in range(2):
                lhsT = combT[ch][:, g * 128:(g + 1) * 128]
                nc.tensor.matmul(psA[:], lhsT, eg[ch][:, 0:512],
                                 start=(ch == 0), stop=(ch == 1))
                nc.tensor.matmul(psB[:], lhsT, eg[ch][:, 512:1024],
                                 start=(ch == 0), stop=(ch == 1))
            ot = p_ot.tile([128, D], BF16, tag="ot", name="ot")
            nc.vector.tensor_tensor(ot[:, 0:512], psA[:], shg[:, 0:512],
                                    op=ALU.add)
            nc.vector.tensor_tensor(ot[:, 512:1024], psB[:],
                                    shg[:, 512:1024], op=ALU.add)
            nc.gpsimd.dma_start(out_d[g * 128:(g + 1) * 128, :], ot[:])

        # ---------- routed expert units ----------
        eo_g = [t[:].rearrange("(g ec) d -> g ec d", ec=128)
                for t in eo_dram]
        for e in range(E):
            def rhs_fn(dc, e=e):
                r = dispT[dc][:].rearrange("p (g ec) -> p g ec", g=NG)
                return r[:, :, e * CAP:(e + 1) * CAP]
            def out_ap_fn(sc, e=e):
                # FFN2 tile rows are slots (g-major): row r -> group
                # sc*4 + r//32, capacity slot r%32 of expert e
                return eo_g[e // 4][sc * 4:(sc + 1) * 4,
                                    (e % 4) * CAP:(e % 4 + 1) * CAP, :]
            nka = k_d[e + 1] if e < E - 1 else sk_d
            _, kbn = _emit_ffn_unit(nc, pools, rhs_fn=rhs_fn,
                                    out_ap_fn=out_ap_fn, keys_ap=k_d[e],
                                    vals_ap=v_d[e], kb_pre=kbn,
                                    next_keys_ap=nka)

        # last shared unit runs after the experts with the combine woven in:
        # each group's expert outputs prefetch a few FFN1 blocks ahead and its
        # 4 combine matmuls fill the PSUM-free windows between FFN1 blocks.
        # No prefetch at weave(0): an eg DMA still waiting on expert 7's eo
        # writes would head-of-line block SH3's keys stream on sync.
        def sh3_weave(b):
            if b == 0:
                return
            if b >= 2:
                cb_compute(b - 2)
            pf = {1: (0, 1), 2: (2, 3), 3: (4, 5), 4: (6, 7),
                  5: (8,), 6: (9,), 7: (10,)}[b]
            for g in pf:
                cb_prefetch(g)

        def sh3_post_ffn1():
            for g in range(6, 10):
                cb_compute(g)
            for g in range(11, 16):
                cb_prefetch(g)

        sh3_eo, _ = emit_shared(3, weave=sh3_weave, post_ffn1=sh3_post_ffn1,
                                kb_pre=kbn)
        for g in range(10, 12):
            cb_compute(g)
        for g in range(12, NG):
            cb_compute(g, shg_override=sh3_eo[g - 12])

        es_cb.close()
        es_ffn.close()
        es_xtb.close()
        es_dT.close()

    nc.compile()
    return nc


LAST_RESULT = None


def kernel(x, gate_weight, gate_bias, keys, values, shared_keys,
           shared_values, **_ignored):
    global LAST_RESULT
    _ensure_ntff_hook()
    BF = ml_dtypes.bfloat16
    x = np.ascontiguousarray(np.asarray(x, dtype=np.float32))
    gate_weight = np.ascontiguousarray(np.asarray(gate_weight, np.float32))
    gate_bias = np.ascontiguousarray(
        np.asarray(gate_bias, np.float32).reshape(1, E))
    keys = np.ascontiguousarray(np.asarray(keys, np.float32).astype(BF))
    values = np.ascontiguousarray(np.asarray(values, np.float32).astype(BF))
    shared_keys = np.ascontiguousarray(
        np.asarray(shared_keys, np.float32).reshape(D, H).astype(BF))
    shared_values = np.ascontiguousarray(
        np.asarray(shared_values, np.float32).reshape(H, D).astype(BF))

    if "nc" not in _CACHE:
        _CACHE["nc"] = _build_program()
    nc = _CACHE["nc"]

    xt = x.reshape(NCORES, TOK, D)
    in_maps = []
    for i in range(NCORES):
        in_maps.append({
            "x_s": np.ascontiguousarray(xt[i]),
            "gw": gate_weight,
            "gb": gate_bias,
            "keys": keys,
            "values": values,
            "shk": shared_keys,
            "shv": shared_values,
        })
    res = run_bass_kernel_spmd(nc, in_maps, core_ids=list(range(NCORES)))
    LAST_RESULT = res
    out = np.concatenate([res.results[i]["out"] for i in range(NCORES)],
                         axis=0)
    return out.reshape(B, S, D).astype(np.float32)


# revision 45
# speedup vs baseline: 1.0089x; 1.0001x over previous
"""Trainium2 Bass kernel for nn_MixtureLayer (MoE routing, 8 experts, top-2,
grouped capacity routing + shared expert).

Strategy: data-parallel over the 128 token-groups -> 16 groups per core.
Each core runs the router, dispatch, all 8 experts' FFNs on its own groups,
the shared expert, and the combine.  No collectives needed.

Weights are pre-cast to bf16 on the host (they are consumed exactly once on
device, so an on-device cast would only add DMA + vector work).  The router
(logits/softmax/top-k/cumsum) runs entirely in fp32 so expert selection
matches the jax reference; the FFN matmuls run bf16 with fp32 PSUM.

Schedule: the 16 router groups are processed in 4 quarters, software-
pipelined with the 4 shared-expert FFN units (which depend only on x^T, not
on routing) so the PE array stays busy during the router's serial DVE
chains.  The 8 routed-expert units follow, and the combine is pipelined
directly behind the last expert's FFN2.
"""

import sys
import types

import numpy as np
import ml_dtypes

try:  # concourse is normally on sys.path via the container's site setup
    import concourse.bass as bass  # noqa: F401
except ImportError:  # pragma: no cover
    sys.path.insert(0, "/opt/trn_rl_repo")

import concourse.bass as bass
import concourse.tile as tile
from concourse import bacc, mybir
from concourse.bass_utils import run_bass_kernel_spmd

F32 = mybir.dt.float32
BF16 = mybir.dt.bfloat16
AF = mybir.ActivationFunctionType
ALU = mybir.AluOpType
GELU = AF.Gelu_apprx_tanh  # jax.nn.gelu(approximate=True)

# ---- problem constants (hardcoded from the spec) ----
NCORES = 8
D, H, E = 1024, 4096, 8
B, S = 8, 2048
GRP = 128                 # tokens per routing group
NG_TOT = 128              # total groups
NG = NG_TOT // NCORES     # groups per core = 16
TOK = NG * GRP            # tokens per core = 2048
CAP = 32                  # capacity slots per (group, expert); slot 31 unused
DC = D // 128             # 8 chunks of d
HC = H // 128             # 32 chunks of h
SLOTS = NG * CAP          # 512 slots per expert per core

_CACHE = {}


def _ensure_ntff_hook():
    """Register the axon NTFF profiling hook if the image's antenv stub lacks
    it (needed only when tracing; harmless otherwise)."""
    try:
        import antenv
    except ImportError:
        return
    if "antenv.axon_hooks" in sys.modules:
        return
    m = types.ModuleType("antenv.axon_hooks")
    m._hook = None

    def _set(h, _m=m):
        _m._hook = h

    def _get(_m=m):
        return _m._hook

    m.set_axon_ntff_profile_hook = _set
    m.get_axon_ntff_profile_hook = _get
    sys.modules["antenv.axon_hooks"] = m
    antenv.axon_hooks = m
    try:
        from trn_agent_boot.trn_boot import _ntff_profile_via_ctypes

        hook = _ntff_profile_via_ctypes("/opt/axon/libaxon_pjrt.so")
        if hook is not None:
            _set(hook)
    except Exception:
        pass


def _emit_ffn_unit(nc, pools, rhs_fn, out_ap_fn, keys_ap, vals_ap,
                   weave=None, post_ffn1=None, kb_pre=None,
                   next_keys_ap=None):
    """One FFN 'unit': 512 input columns (slots/tokens) through d->h gelu h->d.

    rhs_fn(dc) -> AP [128, 512] of the input in transposed layout (d on
    partitions).  keys_ap [D, H] / vals_ap [H, D] are bf16 DRAM APs.
    weave(b) is called after FFN1 block b's gelus (PSUM has >=4 free banks
    there); post_ffn1() after the last block, before FFN2 claims all 8.
    kb_pre: hcb0's 8 keys tiles preloaded by the previous unit.  If
    next_keys_ap is given, this unit preloads the next unit's hcb0 keys
    during its FFN2 and returns them — otherwise each unit's FFN1 start
    stalls ~5us on the keys stream spinning up (the scheduler does not
    hoist a unit's first kb DMAs across the previous unit's FFN2).
    """
    hid = [pools["hid"].tile([128, 512], BF16, tag=f"hid{hc}", name=f"hid{hc}") for hc in range(HC)]
    # FFN1: hid[hc][128, 512] = gelu(sum_dc keys[dc,hc].T @ rhs[dc])
    # The first 8 values tiles are requested one-per-block from inside FFN1
    # (exactly the vb ring depth, so the doorbells never block the gelus) —
    # otherwise FFN2 stalls several us at its start waiting for the values
    # stream to spin up.
    vb_pre = []
    for hcb in range(8):  # blocks of 4 h-chunks
        eps = [pools["ps"].tile([128, 512], F32, tag="ps", name="ps") for _ in range(4)]
        for dc in range(DC):
            if hcb == 0 and kb_pre is not None:
                kb = kb_pre[dc]
            else:
                kb = pools["kb"].tile([128, 512], BF16, tag="kb", name="kb")
                nc.sync.dma_start(kb[:], keys_ap[dc * 128:(dc + 1) * 128,
                                                 hcb * 512:(hcb + 1) * 512])
            rhs = rhs_fn(dc)
            for hh in range(4):
                nc.tensor.matmul(eps[hh][:], kb[:, hh * 128:(hh + 1) * 128], rhs,
                                 start=(dc == 0), stop=(dc == DC - 1))
        for hh in range(4):
            nc.scalar.activation(hid[hcb * 4 + hh][:], eps[hh][:], GELU)
        vb = pools["vb"].tile([128, 1024], BF16, tag="vb", name="vb")
        nc.scalar.dma_start(vb[:], vals_ap[hcb * 128:(hcb + 1) * 128, :])
        vb_pre.append(vb)
        if weave is not None:
            weave(hcb)
    if post_ffn1 is not None:
        post_ffn1()
    # FFN2: out[sc*128.., 1024] = sum_hc hid[hc][:,sc].T @ values[hc]
    # values stream on the scalar (Activation) HWDGE queue so they are not
    # head-of-line blocked behind the keys stream on sync
    pss = [[pools["ps"].tile([128, 512], F32, tag="ps", name="ps") for _ in range(2)]
           for _ in range(4)]
    kb_next = None
    for hc in range(HC):
        if hc < 8:
            vb = vb_pre[hc]
        else:
            vb = pools["vb"].tile([128, 1024], BF16, tag="vb", name="vb")
            # last tiles via sync (idle by then) so the ACT queue reaches
            # this unit's PSUM drains the moment FFN2 stops — the next
            # unit's FFN1 start waits on those drains via the PSUM ring
            q = nc.sync if hc >= 24 else nc.scalar
            q.dma_start(vb[:], vals_ap[hc * 128:(hc + 1) * 128, :])
        if hc == 20 and next_keys_ap is not None:
            kb_next = []
            for dc in range(DC):
                kb = pools["kb"].tile([128, 512], BF16, tag="kb", name="kb")
                nc.sync.dma_start(kb[:],
                                  next_keys_ap[dc * 128:(dc + 1) * 128,
                                               0:512])
                kb_next.append(kb)
        for sc in range(4):
            lhsT = hid[hc][:, sc * 128:(sc + 1) * 128]
            nc.tensor.matmul(pss[sc][0][:], lhsT, vb[:, 0:512],
                             start=(hc == 0), stop=(hc == HC - 1))
            nc.tensor.matmul(pss[sc][1][:], lhsT, vb[:, 512:1024],
                             start=(hc == 0), stop=(hc == HC - 1))
    # drain PSUM on both scalar and vector so the banks free up fast for
    # the next unit's FFN1 accumulators
    eo_tiles = []
    for sc in range(4):
        eo = pools["eo"].tile([128, 1024], BF16, tag="eo", name="eo")
        nc.scalar.copy(eo[:, 0:512], pss[sc][0][:])
        nc.vector.tensor_copy(eo[:, 512:1024], pss[sc][1][:])
        if out_ap_fn is not None:
            nc.gpsimd.dma_start(out_ap_fn(sc), eo[:])
        eo_tiles.append(eo)
    return eo_tiles, kb_next


def _build_program():
    nc = bacc.Bacc("TRN2", target_bir_lowering=False, debug=False,
                   num_devices=NCORES)

    x_d = nc.dram_tensor("x_s", [TOK, D], F32, kind="ExternalInput").ap()
    gw_d = nc.dram_tensor("gw", [D, E], F32, kind="ExternalInput").ap()
    gb_d = nc.dram_tensor("gb", [1, E], F32, kind="ExternalInput").ap()
    k_d = nc.dram_tensor("keys", [E, D, H], BF16, kind="ExternalInput").ap()
    v_d = nc.dram_tensor("values", [E, H, D], BF16, kind="ExternalInput").ap()
    sk_d = nc.dram_tensor("shk", [D, H], BF16, kind="ExternalInput").ap()
    sv_d = nc.dram_tensor("shv", [H, D], BF16, kind="ExternalInput").ap()
    out_d = nc.dram_tensor("out", [TOK, D], BF16, kind="ExternalOutput").ap()

    from contextlib import ExitStack
    with tile.TileContext(nc) as tc, ExitStack() as es_glob:
        es_dT, es_xtb = ExitStack(), ExitStack()
        es_ffn, es_rt, es_cb = ExitStack(), ExitStack(), ExitStack()

        def mk(es, name, bufs, space="SBUF"):
            return es.enter_context(tc.tile_pool(name=name, bufs=bufs,
                                                 space=space))

        # global pools (live for whole kernel)
        ps = mk(es_glob, "ps", 8, "PSUM")
        const = mk(es_glob, "const", 1)
        dram = mk(es_glob, "dram", 1, "DRAM")
        p_ct = mk(es_glob, "p_ct", 1)
        pools = {"ps": ps}

        # ---------- constants ----------
        ones128 = const.tile([128, 128], F32, tag="ones128", name="ones128")
        nc.vector.memset(ones128[:], 1.0)
        ident = const.tile([128, 128], F32, tag="ident", name="ident")
        nc.gpsimd.affine_select(ident[:], ones128[:], pattern=[[1, 128]],
                                base=0, channel_multiplier=-1,
                                compare_op=ALU.is_equal, fill=0.0)
        utri = const.tile([128, 128], F32, tag="utri", name="utri")
        nc.gpsimd.affine_select(utri[:], ones128[:], pattern=[[1, 128]],
                                base=0, channel_multiplier=-1,
                                compare_op=ALU.is_ge, fill=0.0)
        # iota over capacity slots: value c+1 at slot c (c<31), -1 at c=31
        iota_f = const.tile([128, E * CAP], F32, tag="iota_f", name="iota_f")
        nc.gpsimd.iota(iota_f[:], pattern=[[0, E], [1, CAP]], base=1,
                       channel_multiplier=0,
                       allow_small_or_imprecise_dtypes=True)
        iota_3d = iota_f[:].rearrange("p (e c) -> p e c", e=E)
        nc.vector.memset(iota_3d[:, :, CAP - 1:CAP], -1.0)
        # gw/gb DMAs are issued inside emit_A(0), ordered around the first
        # x-group loads on sync, so neither the first transposes nor the
        # first logits matmuls wait on a cold queue
        gw_sb = const.tile([128, DC * E], F32, tag="gw", name="gw")
        gb_sb = const.tile([1, E], F32, tag="gb", name="gb")
        ones1 = const.tile([1, 128], F32, tag="ones1", name="ones1")
        nc.vector.memset(ones1[:], 1.0)
        gbb = const.tile([128, E], F32, tag="gbb", name="gbb")

        # ---------- persistent tensors ----------
        p_dT = mk(es_dT, "p_dT", 1)
        # xTb lives per-quarter (512 tokens) in a depth-2 ring: shared FFN
        # unit q consumes quarter q shortly after the router writes it
        p_xtb = mk(es_xtb, "p_xtb", 2)
        xTb_q = [None] * 4  # quarter -> [DC] tiles of [128, 512] bf16
        combT = [p_ct.tile([128, NG * 128], BF16, tag=f"ct{ch}",
                           name=f"ct{ch}") for ch in range(2)]
        dispT = [p_dT.tile([128, NG * E * CAP], BF16, tag=f"dT{dc}",
                           name=f"dT{dc}") for dc in range(DC)]

        # DRAM scratch
        eo_dram = [dram.tile([NG * 128, D], BF16, tag=f"eo_dram{h}",
                             name=f"eo_dram{h}") for h in range(2)]
        sh_dram = dram.tile([TOK, D], BF16, tag="sh_dram", name="sh_dram")

        # ---------- FFN pools (open early: shared units interleave) ----------
        pools["kb"] = mk(es_ffn, "p_kb", 10)
        pools["vb"] = mk(es_ffn, "p_vb", 8)
        pools["hid"] = mk(es_ffn, "p_hid", 1)
        pools["eo"] = mk(es_ffn, "p_eo", 4)

        # ---------- router pools ----------
        p_xg = mk(es_rt, "p_xg", 4)
        p_xgb = mk(es_rt, "p_xgb", 7)
        p_xtf = mk(es_rt, "p_xtf", 6)
        p_sm = mk(es_rt, "p_sm", 24)
        p_sm8 = mk(es_rt, "p_sm8", 32)
        p_cmp = mk(es_rt, "p_cmp", 3)
        p_cb = mk(es_rt, "p_cb", 6)
        p_dm = mk(es_rt, "p_dm", 6)

        # per-group state carried between pipeline stages
        st_xgb = [None] * NG
        st_xtf = [None] * NG   # [2] tiles of [128,512] f32
        st_lg = [None] * NG    # logits in SBUF f32 [128, E]
        st_mask = [None] * NG  # (mask1, mask2)
        st_m = [None] * NG     # (m1, m2)
        st_dm = [None] * NG    # dispatch mask bf16 [128, E*CAP]
        st_comb = [None] * NG  # combine weights f32 [128, E*CAP]

        def emit_A(q):
            """Per-group: x DMA, bf16 cast, transposes, xTb/xtf, logits."""
            gs = range(4 * q, 4 * q + 4)
            xTb_q[q] = [p_xtb.tile([128, 512], BF16, tag=f"xtb{dc}",
                                   name=f"xtb{dc}") for dc in range(DC)]
            xTb = [t[:] for t in xTb_q[q]]
            for g in gs:
                xh = []
                for half in range(2):
                    t = p_xg.tile([128, 512], F32, tag="xg", name="xg")
                    cols = slice(half * 512, (half + 1) * 512)
                    if g == 0:
                        for piece in range(2):
                            c0 = half * 512 + piece * 256
                            nc.sync.dma_start(
                                t[:, piece * 256:(piece + 1) * 256],
                                x_d[g * 128:(g + 1) * 128, c0:c0 + 256])
                    else:
                        nc.sync.dma_start(t[:],
                                          x_d[g * 128:(g + 1) * 128, cols])
                    xh.append(t)
                if q == 0 and g == 0:
                    # router-weight loads right behind the first x group;
                    # gw as a single multi-fragment DMA
                    nc.sync.dma_start(gb_sb[:], gb_d[:])
                    nc.sync.dma_start(
                        gw_sb[:].rearrange("p (dc e) -> p dc e", dc=DC),
                        gw_d[:].rearrange("(dc p) e -> p dc e", p=128))
                    gbp = ps.tile([128, E], F32, tag="ps", name="ps")
                    nc.tensor.matmul(gbp[:], ones1[:], gb_sb[:],
                                     start=True, stop=True)
                    nc.vector.tensor_copy(gbb[:], gbp[:])
                xgb = p_xgb.tile([128, D], BF16, tag="xgb", name="xgb")
                nc.scalar.copy(xgb[:, 0:512], xh[0][:])
                nc.scalar.copy(xgb[:, 512:1024], xh[1][:])
                st_xgb[g] = xgb
                xtf = []
                for dc4 in range(2):
                    tp = ps.tile([128, 512], F32, tag="ps", name="ps")
                    for j in range(4):
                        nc.tensor.transpose(tp[:, j * 128:(j + 1) * 128],
                                            xh[dc4][:, j * 128:(j + 1) * 128],
                                            ident[:])
                    t = p_xtf.tile([128, 512], F32, tag="xtf", name="xtf")
                    nc.vector.tensor_copy(t[:], tp[:])
                    xtf.append(t)
                    gl = g - 4 * q
                    for j in range(4):
                        dc = dc4 * 4 + j
                        nc.scalar.copy(xTb[dc][:, gl * 128:(gl + 1) * 128],
                                       tp[:, j * 128:(j + 1) * 128])
                st_xtf[g] = xtf
            # logits for the quarter into one PSUM tile, then to SBUF
            lgp = ps.tile([128, 4 * E], F32, tag="ps", name="ps")
            for j, g in enumerate(gs):
                logits = lgp[:, j * E:(j + 1) * E]
                xtf = st_xtf[g]
                for dc in range(DC):
                    nc.tensor.matmul(logits,
                                     xtf[dc // 4][:, (dc % 4) * 128:
                                                  (dc % 4 + 1) * 128],
                                     gw_sb[:, dc * E:(dc + 1) * E],
                                     start=(dc == 0), stop=(dc == DC - 1))
                lg = p_sm8.tile([128, E], F32, tag="sm8", name="lg")
                nc.vector.tensor_tensor(lg[:], logits, gbb[:], op=ALU.add)
                st_lg[g] = lg

        def emit_B(q):
            """Softmax + top-2 masks (DVE/ACT only)."""
            for g in range(4 * q, 4 * q + 4):
                lg = st_lg[g]
                negm = p_sm.tile([128, 1], F32, tag="sm", name="negm")
                nc.vector.tensor_reduce(negm[:], lg[:],
                                        axis=mybir.AxisListType.X,
                                        op=ALU.max, negate=True)
                ex = p_sm8.tile([128, E], F32, tag="sm8", name="ex")
                den = p_sm.tile([128, 1], F32, tag="sm", name="den")
                nc.scalar.activation(ex[:], lg[:], AF.Exp, bias=negm[:],
                                     scale=1.0, accum_out=den[:])
                rec = p_sm.tile([128, 1], F32, tag="sm", name="rec")
                nc.vector.reciprocal(rec[:], den[:])
                probs = p_sm8.tile([128, E], F32, tag="sm8", name="probs")
                nc.vector.tensor_scalar_mul(probs[:], ex[:], rec[:])
                m1 = p_sm.tile([128, 1], F32, tag="sm", name="m1")
                nc.vector.reduce_max(m1[:], probs[:],
                                     axis=mybir.AxisListType.X)
                mask1 = p_sm8.tile([128, E], F32, tag="sm8", name="mask1")
                nc.vector.tensor_scalar(mask1[:], probs[:], m1[:], None,
                                        op0=ALU.is_ge)
                probs2 = p_sm8.tile([128, E], F32, tag="sm8", name="probs2")
                nc.vector.scalar_tensor_tensor(probs2[:], mask1[:], -1e30,
                                               probs[:], ALU.mult, ALU.add)
                m2 = p_sm.tile([128, 1], F32, tag="sm", name="m2")
                nc.vector.reduce_max(m2[:], probs2[:],
                                     axis=mybir.AxisListType.X)
                mask2 = p_sm8.tile([128, E], F32, tag="sm8", name="mask2")
                nc.vector.tensor_scalar(mask2[:], probs2[:], m2[:], None,
                                        op0=ALU.is_ge)
                st_mask[g] = (mask1, mask2)
                st_m[g] = (m1, m2)

        def emit_C(q):
            """Position cumsums (PE) + capacity compare chain (DVE)."""
            gs = range(4 * q, 4 * q + 4)
            ppq = ps.tile([128, 4 * 2 * E], F32, tag="ps", name="ps")
            for j, g in enumerate(gs):
                for ki in range(2):
                    pp = ppq[:, (j * 2 + ki) * E:(j * 2 + ki + 1) * E]
                    nc.tensor.matmul(pp, utri[:], st_mask[g][ki][:],
                                     start=True, stop=True)
            for j, g in enumerate(gs):
                pos = []
                for ki in range(2):
                    pp = ppq[:, (j * 2 + ki) * E:(j * 2 + ki + 1) * E]
                    pm = p_sm8.tile([128, E], F32, tag="sm8", name="pos")
                    nc.vector.tensor_mul(pm[:], pp, st_mask[g][ki][:])
                    pos.append(pm)
                m1, m2 = st_m[g]
                cmp1 = p_cmp.tile([128, E * CAP], F32, tag="cmp", name="cmp1")
                nc.vector.tensor_tensor(
                    cmp1[:].rearrange("p (e c) -> p e c", e=E),
                    pos[0][:].unsqueeze(2).broadcast_to([128, E, CAP]),
                    iota_3d, op=ALU.is_equal)
                cmp2 = p_cmp.tile([128, E * CAP], F32, tag="cmp", name="cmp2")
                nc.vector.tensor_tensor(
                    cmp2[:].rearrange("p (e c) -> p e c", e=E),
                    pos[1][:].unsqueeze(2).broadcast_to([128, E, CAP]),
                    iota_3d, op=ALU.is_equal)
                dm = p_dm.tile([128, E * CAP], BF16, tag="dm", name="dm")
                nc.vector.tensor_add(dm[:], cmp1[:], cmp2[:])
                st_dm[g] = dm
                cmp2s = p_cmp.tile([128, E * CAP], F32, tag="cmp",
                                   name="cmp2s")
                nc.vector.tensor_scalar_mul(cmp2s[:], cmp2[:], m2[:])
                comb = p_cb.tile([128, E * CAP], F32, tag="cb", name="comb")
                nc.vector.scalar_tensor_tensor(comb[:], cmp1[:], m1[:],
                                               cmp2s[:], ALU.mult, ALU.add)
                st_comb[g] = comb

        def emit_EF(q):
            """Dispatch matmuls + combT transposes (PE) and copies out."""
            for g in range(4 * q, 4 * q + 4):
                xgb, dm, comb = st_xgb[g], st_dm[g], st_comb[g]
                for dcp in range(4):
                    dps = ps.tile([128, 512], F32, tag="ps", name="ps")
                    for j in range(2):
                        dc = dcp * 2 + j
                        nc.tensor.matmul(dps[:, j * 256:(j + 1) * 256],
                                         xgb[:, dc * 128:(dc + 1) * 128],
                                         dm[:], start=True, stop=True)
                    # drain on both PSUM-capable engines so the ring keeps
                    # up with the dispatch matmuls (gpsimd cannot read PSUM)
                    for j in range(2):
                        dc = dcp * 2 + j
                        dst = dispT[dc][:, g * E * CAP:(g + 1) * E * CAP]
                        if j == 0:
                            nc.vector.tensor_copy(dst, dps[:, 0:256])
                        else:
                            nc.scalar.copy(dst, dps[:, 256:512])
                ctp = ps.tile([128, 256], F32, tag="ps", name="ps")
                for ch in range(2):
                    nc.tensor.transpose(ctp[:, ch * 128:(ch + 1) * 128],
                                        comb[:, ch * 128:(ch + 1) * 128],
                                        ident[:])
                for ch in range(2):
                    nc.vector.tensor_copy(combT[ch][:, g * 128:(g + 1) * 128],
                                          ctp[:, ch * 128:(ch + 1) * 128])

        def emit_shared(q, **kw):
            # q3's shared outputs are consumed straight from SBUF by the
            # combine; the other quarters roundtrip through sh_dram
            oaf = None
            if q < 3:
                def oaf(sc, q=q):
                    return sh_dram[q * 512 + sc * 128:
                                   q * 512 + (sc + 1) * 128, :]
            return _emit_ffn_unit(
                nc, pools,
                rhs_fn=lambda dc, q=q: xTb_q[q][dc][:],
                out_ap_fn=oaf, keys_ap=sk_d, vals_ap=sv_d, **kw)

        # ---------- software-pipelined router + shared FFN ----------
        emit_A(0); emit_B(0); emit_C(0)
        emit_A(1); emit_B(1)
        _, kbn = emit_shared(0, next_keys_ap=sk_d)
        emit_EF(0); emit_C(1)
        emit_A(2); emit_B(2)
        _, kbn = emit_shared(1, kb_pre=kbn, next_keys_ap=sk_d)
        emit_EF(1); emit_C(2)
        emit_A(3); emit_B(3)
        _, kbn = emit_shared(2, kb_pre=kbn, next_keys_ap=k_d[0])
        emit_EF(2); emit_C(3); emit_EF(3)
        es_rt.close()

        # ---------- combine pools (open in freed router space; LIFO: close
        # before the ffn pools) ----------
        p_eg = mk(es_cb, "p_eg", 12)
        p_shg = mk(es_cb, "p_shg", 8)
        p_ot = mk(es_cb, "p_ot", 3)
        cb_state = [None] * NG

        def cb_prefetch(g):
            eg = [p_eg.tile([128, D], BF16, tag="eg", name="eg")
                  for _ in range(2)]
            for ch in range(2):
                nc.sync.dma_start(eg[ch][:],
                                  eo_dram[ch][g * 128:(g + 1) * 128, :])
            if g < 12:
                shg = p_shg.tile([128, D], BF16, tag="shg", name="shg")
                nc.scalar.dma_start(shg[:],
                                    sh_dram[g * 128:(g + 1) * 128, :])
            else:
                shg = None  # groups 12-15 add SH3's eo tiles straight from SBUF
            cb_state[g] = (eg, shg)

        def cb_compute(g, shg_override=None):
            eg, shg = cb_state[g]
            if shg_override is not None:
                shg = shg_override
            psA = ps.tile([128, 512], F32, tag="ps", name="ps")
            psB = ps.tile([128, 512], F32, tag="ps", name="ps")
            for ch in range(2):
                lhsT = combT[ch][:, g * 128:(g + 1) * 128]
                nc.tensor.matmul(psA[:], lhsT, eg[ch][:, 0:512],
                                 start=(ch == 0), stop=(ch == 1))
                nc.tensor.matmul(psB[:], lhsT, eg[ch][:, 512:1024],
                                 start=(ch == 0), stop=(ch == 1))
            ot = p_ot.tile([128, D], BF16, tag="ot", name="ot")
            nc.vector.tensor_tensor(ot[:, 0:512], psA[:], shg[:, 0:512],
                                    op=ALU.add)
            nc.vector.tensor_tensor(ot[:, 512:1024], psB[:],
                                    shg[:, 512:1024], op=ALU.add)
            nc.gpsimd.dma_start(out_d[g * 128:(g + 1) * 128, :], ot[:])

        # ---------- routed expert units ----------
        eo_g = [t[:].rearrange("(g ec) d -> g ec d", ec=128)
                for t in eo_dram]
        for e in range(E):
            def rhs_fn(dc, e=e):
                r = dispT[dc][:].rearrange("p (g ec) -> p g ec", g=NG)
                return r[:, :, e * CAP:(e + 1) * CAP]
            def out_ap_fn(sc, e=e):
                # FFN2 tile rows are slots (g-major): row r -> group
                # sc*4 + r//32, capacity slot r%32 of expert e
                return eo_g[e // 4][sc * 4:(sc + 1) * 4,
                                    (e % 4) * CAP:(e % 4 + 1) * CAP, :]
            nka = k_d[e + 1] if e < E - 1 else sk_d
            _, kbn = _emit_ffn_unit(nc, pools, rhs_fn=rhs_fn,
                                    out_ap_fn=out_ap_fn, keys_ap=k_d[e],
                                    vals_ap=v_d[e], kb_pre=kbn,
                                    next_keys_ap=nka)

        # last shared unit runs after the experts with the combine woven in:
        # each group's expert outputs prefetch a few FFN1 blocks ahead and its
        # 4 combine matmuls fill the PSUM-free windows between FFN1 blocks.
        # No prefetch at weave(0): an eg DMA still waiting on expert 7's eo
        # writes would head-of-line block SH3's keys stream on sync.
        def sh3_weave(b):
            if b == 0:
                return
            if b >= 2:
                cb_compute(b - 2)
            pf = {1: (0, 1), 2: (2, 3), 3: (4, 5), 4: (6, 7),
                  5: (8,), 6: (9,), 7: (10,)}[b]
            for g in pf:
                cb_prefetch(g)

        def sh3_post_ffn1():
            for g in range(6, 10):
                cb_compute(g)
            for g in range(11, 16):
                cb_prefetch(g)

        sh3_eo, _ = emit_shared(3, weave=sh3_weave, post_ffn1=sh3_post_ffn1,
                                kb_pre=kbn)
        for g in range(10, 12):
            cb_compute(g)
        for g in range(12, NG):
            cb_compute(g, shg_override=sh3_eo[g - 12])

        es_cb.close()
        es_ffn.close()
        es_xtb.close()
        es_dT.close()

    nc.compile()
    return nc


LAST_RESULT = None


def kernel(x, gate_weight, gate_bias, keys, values, shared_keys,
           shared_values, **_ignored):
    global LAST_RESULT
    _ensure_ntff_hook()
    BF = ml_dtypes.bfloat16
    x = np.ascontiguousarray(np.asarray(x, dtype=np.float32))
    gate_weight = np.ascontiguousarray(np.asarray(gate_weight, np.float32))
    gate_bias = np.ascontiguousarray(
        np.asarray(gate_bias, np.float32).reshape(1, E))
    keys = np.ascontiguousarray(np.asarray(keys, np.float32).astype(BF))
    values = np.ascontiguousarray(np.asarray(values, np.float32).astype(BF))
    shared_keys = np.ascontiguousarray(
        np.asarray(shared_keys, np.float32).reshape(D, H).astype(BF))
    shared_values = np.ascontiguousarray(
        np.asarray(shared_values, np.float32).reshape(H, D).astype(BF))

    if "nc" not in _CACHE:
        _CACHE["nc"] = _build_program()
    nc = _CACHE["nc"]

    xt = x.reshape(NCORES, TOK, D)
    in_maps = []
    for i in range(NCORES):
        in_maps.append({
            "x_s": np.ascontiguousarray(xt[i]),
            "gw": gate_weight,
            "gb": gate_bias,
            "keys": keys,
            "values": values,
            "shk": shared_keys,
            "shv": shared_values,
        })
    res = run_bass_kernel_spmd(nc, in_maps, core_ids=list(range(NCORES)))
    LAST_RESULT = res
    out = np.concatenate([res.results[i]["out"] for i in range(NCORES)],
                         axis=0)
    return out.reshape(B, S, D).astype(np.float32)


# revision 47
# speedup vs baseline: 1.0123x; 1.0033x over previous
"""Trainium2 Bass kernel for nn_MixtureLayer (MoE routing, 8 experts, top-2,
grouped capacity routing + shared expert).

Strategy: data-parallel over the 128 token-groups -> 16 groups per core.
Each core runs the router, dispatch, all 8 experts' FFNs on its own groups,
the shared expert, and the combine.  No collectives needed.

Weights are pre-cast to bf16 on the host (they are consumed exactly once on
device, so an on-device cast would only add DMA + vector work).  The router
(logits/softmax/top-k/cumsum) runs entirely in fp32 so expert selection
matches the jax reference; the FFN matmuls run bf16 with fp32 PSUM.

Schedule: the 16 router groups are processed in 4 quarters, software-
pipelined with the 4 shared-expert FFN units (which depend only on x^T, not
on routing) so the PE array stays busy during the router's serial DVE
chains.  The 8 routed-expert units follow, and the combine is pipelined
directly behind the last expert's FFN2.
"""

import sys
import types

import numpy as np
import ml_dtypes

try:  # concourse is normally on sys.path via the container's site setup
    import concourse.bass as bass  # noqa: F401
except ImportError:  # pragma: no cover
    sys.path.insert(0, "/opt/trn_rl_repo")

import concourse.bass as bass
import concourse.tile as tile
from concourse import bacc, mybir
from concourse.bass_utils import run_bass_kernel_spmd

F32 = mybir.dt.float32
BF16 = mybir.dt.bfloat16
AF = mybir.ActivationFunctionType
ALU = mybir.AluOpType
GELU = AF.Gelu_apprx_tanh  # jax.nn.gelu(approximate=True)

# ---- problem constants (hardcoded from the spec) ----
NCORES = 8
D, H, E = 1024, 4096, 8
B, S = 8, 2048
GRP = 128                 # tokens per routing group
NG_TOT = 128              # total groups
NG = NG_TOT // NCORES     # groups per core = 16
TOK = NG * GRP            # tokens per core = 2048
CAP = 32                  # capacity slots per (group, expert); slot 31 unused
DC = D // 128             # 8 chunks of d
HC = H // 128             # 32 chunks of h
SLOTS = NG * CAP          # 512 slots per expert per core

_CACHE = {}


def _ensure_ntff_hook():
    """Register the axon NTFF profiling hook if the image's antenv stub lacks
    it (needed only when tracing; harmless otherwise)."""
    try:
        import antenv
    except ImportError:
        return
    if "antenv.axon_hooks" in sys.modules:
        return
    m = types.ModuleType("antenv.axon_hooks")
    m._hook = None

    def _set(h, _m=m):
        _m._hook = h

    def _get(_m=m):
        return _m._hook

    m.set_axon_ntff_profile_hook = _set
    m.get_axon_ntff_profile_hook = _get
    sys.modules["antenv.axon_hooks"] = m
    antenv.axon_hooks = m
    try:
        from trn_agent_boot.trn_boot import _ntff_profile_via_ctypes

        hook = _ntff_profile_via_ctypes("/opt/axon/libaxon_pjrt.so")
        if hook is not None:
            _set(hook)
    except Exception:
        pass


def _emit_ffn_unit(nc, pools, rhs_fn, out_ap_fn, keys_ap, vals_ap,
                   weave=None, post_ffn1=None, kb_pre=None,
                   next_keys_ap=None):
    """One FFN 'unit': 512 input columns (slots/tokens) through d->h gelu h->d.

    rhs_fn(dc) -> AP [128, 512] of the input in transposed layout (d on
    partitions).  keys_ap [D, H] / vals_ap [H, D] are bf16 DRAM APs.
    weave(b) is called after FFN1 block b's gelus (PSUM has >=4 free banks
    there); post_ffn1() after the last block, before FFN2 claims all 8.
    kb_pre: hcb0's 8 keys tiles preloaded by the previous unit.  If
    next_keys_ap is given, this unit preloads the next unit's hcb0 keys
    during its FFN2 and returns them — otherwise each unit's FFN1 start
    stalls ~5us on the keys stream spinning up (the scheduler does not
    hoist a unit's first kb DMAs across the previous unit's FFN2).
    """
    hid = [pools["hid"].tile([128, 512], BF16, tag=f"hid{hc}", name=f"hid{hc}") for hc in range(HC)]
    # FFN1: hid[hc][128, 512] = gelu(sum_dc keys[dc,hc].T @ rhs[dc])
    # The first 8 values tiles are requested one-per-block from inside FFN1
    # (exactly the vb ring depth, so the doorbells never block the gelus) —
    # otherwise FFN2 stalls several us at its start waiting for the values
    # stream to spin up.
    vb_pre = []
    for hcb in range(8):  # blocks of 4 h-chunks
        eps = [pools["ps"].tile([128, 512], F32, tag="ps", name="ps") for _ in range(4)]
        for dc in range(DC):
            if hcb == 0 and kb_pre is not None:
                kb = kb_pre[dc]
            else:
                kb = pools["kb"].tile([128, 512], BF16, tag="kb", name="kb")
                nc.sync.dma_start(kb[:], keys_ap[dc * 128:(dc + 1) * 128,
                                                 hcb * 512:(hcb + 1) * 512])
            rhs = rhs_fn(dc)
            for hh in range(4):
                nc.tensor.matmul(eps[hh][:], kb[:, hh * 128:(hh + 1) * 128], rhs,
                                 start=(dc == 0), stop=(dc == DC - 1))
        for hh in range(4):
            nc.scalar.activation(hid[hcb * 4 + hh][:], eps[hh][:], GELU)
        vb = pools["vb"].tile([128, 1024], BF16, tag="vb", name="vb")
        nc.sync.dma_start(vb[:], vals_ap[hcb * 128:(hcb + 1) * 128, :])
        vb_pre.append(vb)
        if weave is not None:
            weave(hcb)
    if post_ffn1 is not None:
        post_ffn1()
    # FFN2: out[sc*128.., 1024] = sum_hc hid[hc][:,sc].T @ values[hc]
    # values stream on the scalar (Activation) HWDGE queue so they are not
    # head-of-line blocked behind the keys stream on sync
    pss = [[pools["ps"].tile([128, 512], F32, tag="ps", name="ps") for _ in range(2)]
           for _ in range(4)]
    kb_next = None
    for hc in range(HC):
        if hc < 8:
            vb = vb_pre[hc]
        else:
            vb = pools["vb"].tile([128, 1024], BF16, tag="vb", name="vb")
            # whole values stream on sync: FFN2's first matmul carries a
            # position-based wait on the ACT queue, so ACT must hold only
            # gelus and PSUM drains, never DMA doorbells
            nc.sync.dma_start(vb[:], vals_ap[hc * 128:(hc + 1) * 128, :])
        if hc == 20 and next_keys_ap is not None:
            kb_next = []
            for dc in range(DC):
                kb = pools["kb"].tile([128, 512], BF16, tag="kb", name="kb")
                nc.sync.dma_start(kb[:],
                                  next_keys_ap[dc * 128:(dc + 1) * 128,
                                               0:512])
                kb_next.append(kb)
        for sc in range(4):
            lhsT = hid[hc][:, sc * 128:(sc + 1) * 128]
            nc.tensor.matmul(pss[sc][0][:], lhsT, vb[:, 0:512],
                             start=(hc == 0), stop=(hc == HC - 1))
            nc.tensor.matmul(pss[sc][1][:], lhsT, vb[:, 512:1024],
                             start=(hc == 0), stop=(hc == HC - 1))
    # drain PSUM on both scalar and vector so the banks free up fast for
    # the next unit's FFN1 accumulators
    eo_tiles = []
    for sc in range(4):
        eo = pools["eo"].tile([128, 1024], BF16, tag="eo", name="eo")
        nc.scalar.copy(eo[:, 0:512], pss[sc][0][:])
        nc.vector.tensor_copy(eo[:, 512:1024], pss[sc][1][:])
        if out_ap_fn is not None:
            nc.gpsimd.dma_start(out_ap_fn(sc), eo[:])
        eo_tiles.append(eo)
    return eo_tiles, kb_next


def _build_program():
    nc = bacc.Bacc("TRN2", target_bir_lowering=False, debug=False,
                   num_devices=NCORES)

    x_d = nc.dram_tensor("x_s", [TOK, D], F32, kind="ExternalInput").ap()
    gw_d = nc.dram_tensor("gw", [D, E], F32, kind="ExternalInput").ap()
    gb_d = nc.dram_tensor("gb", [1, E], F32, kind="ExternalInput").ap()
    k_d = nc.dram_tensor("keys", [E, D, H], BF16, kind="ExternalInput").ap()
    v_d = nc.dram_tensor("values", [E, H, D], BF16, kind="ExternalInput").ap()
    sk_d = nc.dram_tensor("shk", [D, H], BF16, kind="ExternalInput").ap()
    sv_d = nc.dram_tensor("shv", [H, D], BF16, kind="ExternalInput").ap()
    out_d = nc.dram_tensor("out", [TOK, D], BF16, kind="ExternalOutput").ap()

    from contextlib import ExitStack
    with tile.TileContext(nc) as tc, ExitStack() as es_glob:
        es_dT, es_xtb = ExitStack(), ExitStack()
        es_ffn, es_rt, es_cb = ExitStack(), ExitStack(), ExitStack()

        def mk(es, name, bufs, space="SBUF"):
            return es.enter_context(tc.tile_pool(name=name, bufs=bufs,
                                                 space=space))

        # global pools (live for whole kernel)
        ps = mk(es_glob, "ps", 8, "PSUM")
        const = mk(es_glob, "const", 1)
        dram = mk(es_glob, "dram", 1, "DRAM")
        p_ct = mk(es_glob, "p_ct", 1)
        pools = {"ps": ps}

        # ---------- constants ----------
        ones128 = const.tile([128, 128], F32, tag="ones128", name="ones128")
        nc.vector.memset(ones128[:], 1.0)
        ident = const.tile([128, 128], F32, tag="ident", name="ident")
        nc.gpsimd.affine_select(ident[:], ones128[:], pattern=[[1, 128]],
                                base=0, channel_multiplier=-1,
                                compare_op=ALU.is_equal, fill=0.0)
        utri = const.tile([128, 128], F32, tag="utri", name="utri")
        nc.gpsimd.affine_select(utri[:], ones128[:], pattern=[[1, 128]],
                                base=0, channel_multiplier=-1,
                                compare_op=ALU.is_ge, fill=0.0)
        # iota over capacity slots: value c+1 at slot c (c<31), -1 at c=31
        iota_f = const.tile([128, E * CAP], F32, tag="iota_f", name="iota_f")
        nc.gpsimd.iota(iota_f[:], pattern=[[0, E], [1, CAP]], base=1,
                       channel_multiplier=0,
                       allow_small_or_imprecise_dtypes=True)
        iota_3d = iota_f[:].rearrange("p (e c) -> p e c", e=E)
        nc.vector.memset(iota_3d[:, :, CAP - 1:CAP], -1.0)
        # gw/gb DMAs are issued inside emit_A(0), ordered around the first
        # x-group loads on sync, so neither the first transposes nor the
        # first logits matmuls wait on a cold queue
        gw_sb = const.tile([128, DC * E], F32, tag="gw", name="gw")
        gb_sb = const.tile([1, E], F32, tag="gb", name="gb")
        ones1 = const.tile([1, 128], F32, tag="ones1", name="ones1")
        nc.vector.memset(ones1[:], 1.0)
        gbb = const.tile([128, E], F32, tag="gbb", name="gbb")

        # ---------- persistent tensors ----------
        p_dT = mk(es_dT, "p_dT", 1)
        # xTb lives per-quarter (512 tokens) in a depth-2 ring: shared FFN
        # unit q consumes quarter q shortly after the router writes it
        p_xtb = mk(es_xtb, "p_xtb", 2)
        xTb_q = [None] * 4  # quarter -> [DC] tiles of [128, 512] bf16
        combT = [p_ct.tile([128, NG * 128], BF16, tag=f"ct{ch}",
                           name=f"ct{ch}") for ch in range(2)]
        dispT = [p_dT.tile([128, NG * E * CAP], BF16, tag=f"dT{dc}",
                           name=f"dT{dc}") for dc in range(DC)]

        # DRAM scratch
        eo_dram = [dram.tile([NG * 128, D], BF16, tag=f"eo_dram{h}",
                             name=f"eo_dram{h}") for h in range(2)]
        sh_dram = dram.tile([TOK, D], BF16, tag="sh_dram", name="sh_dram")

        # ---------- FFN pools (open early: shared units interleave) ----------
        pools["kb"] = mk(es_ffn, "p_kb", 10)
        pools["vb"] = mk(es_ffn, "p_vb", 8)
        pools["hid"] = mk(es_ffn, "p_hid", 1)
        pools["eo"] = mk(es_ffn, "p_eo", 4)

        # ---------- router pools ----------
        p_xg = mk(es_rt, "p_xg", 4)
        p_xgb = mk(es_rt, "p_xgb", 7)
        p_xtf = mk(es_rt, "p_xtf", 6)
        p_sm = mk(es_rt, "p_sm", 24)
        p_sm8 = mk(es_rt, "p_sm8", 32)
        p_cmp = mk(es_rt, "p_cmp", 3)
        p_cb = mk(es_rt, "p_cb", 6)
        p_dm = mk(es_rt, "p_dm", 6)

        # per-group state carried between pipeline stages
        st_xgb = [None] * NG
        st_xtf = [None] * NG   # [2] tiles of [128,512] f32
        st_lg = [None] * NG    # logits in SBUF f32 [128, E]
        st_mask = [None] * NG  # (mask1, mask2)
        st_m = [None] * NG     # (m1, m2)
        st_dm = [None] * NG    # dispatch mask bf16 [128, E*CAP]
        st_comb = [None] * NG  # combine weights f32 [128, E*CAP]

        def emit_A(q):
            """Per-group: x DMA, bf16 cast, transposes, xTb/xtf, logits."""
            gs = range(4 * q, 4 * q + 4)
            xTb_q[q] = [p_xtb.tile([128, 512], BF16, tag=f"xtb{dc}",
                                   name=f"xtb{dc}") for dc in range(DC)]
            xTb = [t[:] for t in xTb_q[q]]
            for g in gs:
                xh = []
                for half in range(2):
                    t = p_xg.tile([128, 512], F32, tag="xg", name="xg")
                    cols = slice(half * 512, (half + 1) * 512)
                    if g == 0:
                        for piece in range(2):
                            c0 = half * 512 + piece * 256
                            nc.sync.dma_start(
                                t[:, piece * 256:(piece + 1) * 256],
                                x_d[g * 128:(g + 1) * 128, c0:c0 + 256])
                    else:
                        nc.sync.dma_start(t[:],
                                          x_d[g * 128:(g + 1) * 128, cols])
                    xh.append(t)
                if q == 0 and g == 0:
                    # router-weight loads right behind the first x group;
                    # gw as a single multi-fragment DMA
                    nc.sync.dma_start(gb_sb[:], gb_d[:])
                    nc.sync.dma_start(
                        gw_sb[:].rearrange("p (dc e) -> p dc e", dc=DC),
                        gw_d[:].rearrange("(dc p) e -> p dc e", p=128))
                    gbp = ps.tile([128, E], F32, tag="ps", name="ps")
                    nc.tensor.matmul(gbp[:], ones1[:], gb_sb[:],
                                     start=True, stop=True)
                    nc.vector.tensor_copy(gbb[:], gbp[:])
                xgb = p_xgb.tile([128, D], BF16, tag="xgb", name="xgb")
                nc.scalar.copy(xgb[:, 0:512], xh[0][:])
                nc.scalar.copy(xgb[:, 512:1024], xh[1][:])
                st_xgb[g] = xgb
                xtf = []
                for dc4 in range(2):
                    tp = ps.tile([128, 512], F32, tag="ps", name="ps")
                    for j in range(4):
                        nc.tensor.transpose(tp[:, j * 128:(j + 1) * 128],
                                            xh[dc4][:, j * 128:(j + 1) * 128],
                                            ident[:])
                    t = p_xtf.tile([128, 512], F32, tag="xtf", name="xtf")
                    nc.vector.tensor_copy(t[:], tp[:])
                    xtf.append(t)
                    gl = g - 4 * q
                    for j in range(4):
                        dc = dc4 * 4 + j
                        nc.scalar.copy(xTb[dc][:, gl * 128:(gl + 1) * 128],
                                       tp[:, j * 128:(j + 1) * 128])
                st_xtf[g] = xtf
            # logits for the quarter into one PSUM tile, then to SBUF
            lgp = ps.tile([128, 4 * E], F32, tag="ps", name="ps")
            for j, g in enumerate(gs):
                logits = lgp[:, j * E:(j + 1) * E]
                xtf = st_xtf[g]
                for dc in range(DC):
                    nc.tensor.matmul(logits,
                                     xtf[dc // 4][:, (dc % 4) * 128:
                                                  (dc % 4 + 1) * 128],
                                     gw_sb[:, dc * E:(dc + 1) * E],
                                     start=(dc == 0), stop=(dc == DC - 1))
                lg = p_sm8.tile([128, E], F32, tag="sm8", name="lg")
                nc.vector.tensor_tensor(lg[:], logits, gbb[:], op=ALU.add)
                st_lg[g] = lg

        def emit_B(q):
            """Softmax + top-2 masks (DVE/ACT only)."""
            for g in range(4 * q, 4 * q + 4):
                lg = st_lg[g]
                negm = p_sm.tile([128, 1], F32, tag="sm", name="negm")
                nc.vector.tensor_reduce(negm[:], lg[:],
                                        axis=mybir.AxisListType.X,
                                        op=ALU.max, negate=True)
                ex = p_sm8.tile([128, E], F32, tag="sm8", name="ex")
                den = p_sm.tile([128, 1], F32, tag="sm", name="den")
                nc.scalar.activation(ex[:], lg[:], AF.Exp, bias=negm[:],
                                     scale=1.0, accum_out=den[:])
                rec = p_sm.tile([128, 1], F32, tag="sm", name="rec")
                nc.vector.reciprocal(rec[:], den[:])
                probs = p_sm8.tile([128, E], F32, tag="sm8", name="probs")
                nc.vector.tensor_scalar_mul(probs[:], ex[:], rec[:])
                m1 = p_sm.tile([128, 1], F32, tag="sm", name="m1")
                nc.vector.reduce_max(m1[:], probs[:],
                                     axis=mybir.AxisListType.X)
                mask1 = p_sm8.tile([128, E], F32, tag="sm8", name="mask1")
                nc.vector.tensor_scalar(mask1[:], probs[:], m1[:], None,
                                        op0=ALU.is_ge)
                probs2 = p_sm8.tile([128, E], F32, tag="sm8", name="probs2")
                nc.vector.scalar_tensor_tensor(probs2[:], mask1[:], -1e30,
                                               probs[:], ALU.mult, ALU.add)
                m2 = p_sm.tile([128, 1], F32, tag="sm", name="m2")
                nc.vector.reduce_max(m2[:], probs2[:],
                                     axis=mybir.AxisListType.X)
                mask2 = p_sm8.tile([128, E], F32, tag="sm8", name="mask2")
                nc.vector.tensor_scalar(mask2[:], probs2[:], m2[:], None,
                                        op0=ALU.is_ge)
                st_mask[g] = (mask1, mask2)
                st_m[g] = (m1, m2)

        def emit_C(q):
            """Position cumsums (PE) + capacity compare chain (DVE)."""
            gs = range(4 * q, 4 * q + 4)
            ppq = ps.tile([128, 4 * 2 * E], F32, tag="ps", name="ps")
            for j, g in enumerate(gs):
                for ki in range(2):
                    pp = ppq[:, (j * 2 + ki) * E:(j * 2 + ki + 1) * E]
                    nc.tensor.matmul(pp, utri[:], st_mask[g][ki][:],
                                     start=True, stop=True)
            for j, g in enumerate(gs):
                pos = []
                for ki in range(2):
                    pp = ppq[:, (j * 2 + ki) * E:(j * 2 + ki + 1) * E]
                    pm = p_sm8.tile([128, E], F32, tag="sm8", name="pos")
                    nc.vector.tensor_mul(pm[:], pp, st_mask[g][ki][:])
                    pos.append(pm)
                m1, m2 = st_m[g]
                cmp1 = p_cmp.tile([128, E * CAP], F32, tag="cmp", name="cmp1")
                nc.vector.tensor_tensor(
                    cmp1[:].rearrange("p (e c) -> p e c", e=E),
                    pos[0][:].unsqueeze(2).broadcast_to([128, E, CAP]),
                    iota_3d, op=ALU.is_equal)
                cmp2 = p_cmp.tile([128, E * CAP], F32, tag="cmp", name="cmp2")
                nc.vector.tensor_tensor(
                    cmp2[:].rearrange("p (e c) -> p e c", e=E),
                    pos[1][:].unsqueeze(2).broadcast_to([128, E, CAP]),
                    iota_3d, op=ALU.is_equal)
                dm = p_dm.tile([128, E * CAP], BF16, tag="dm", name="dm")
                nc.vector.tensor_add(dm[:], cmp1[:], cmp2[:])
                st_dm[g] = dm
                cmp2s = p_cmp.tile([128, E * CAP], F32, tag="cmp",
                                   name="cmp2s")
                nc.vector.tensor_scalar_mul(cmp2s[:], cmp2[:], m2[:])
                comb = p_cb.tile([128, E * CAP], F32, tag="cb", name="comb")
                nc.vector.scalar_tensor_tensor(comb[:], cmp1[:], m1[:],
                                               cmp2s[:], ALU.mult, ALU.add)
                st_comb[g] = comb

        def emit_EF(q):
            """Dispatch matmuls + combT transposes (PE) and copies out."""
            for g in range(4 * q, 4 * q + 4):
                xgb, dm, comb = st_xgb[g], st_dm[g], st_comb[g]
                for dcp in range(4):
                    dps = ps.tile([128, 512], F32, tag="ps", name="ps")
                    for j in range(2):
                        dc = dcp * 2 + j
                        nc.tensor.matmul(dps[:, j * 256:(j + 1) * 256],
                                         xgb[:, dc * 128:(dc + 1) * 128],
                                         dm[:], start=True, stop=True)
                    # drain on both PSUM-capable engines so the ring keeps
                    # up with the dispatch matmuls (gpsimd cannot read PSUM)
                    for j in range(2):
                        dc = dcp * 2 + j
                        dst = dispT[dc][:, g * E * CAP:(g + 1) * E * CAP]
                        if j == 0:
                            nc.vector.tensor_copy(dst, dps[:, 0:256])
                        else:
                            nc.scalar.copy(dst, dps[:, 256:512])
                ctp = ps.tile([128, 256], F32, tag="ps", name="ps")
                for ch in range(2):
                    nc.tensor.transpose(ctp[:, ch * 128:(ch + 1) * 128],
                                        comb[:, ch * 128:(ch + 1) * 128],
                                        ident[:])
                for ch in range(2):
                    nc.vector.tensor_copy(combT[ch][:, g * 128:(g + 1) * 128],
                                          ctp[:, ch * 128:(ch + 1) * 128])

        def emit_shared(q, **kw):
            # q3's shared outputs are consumed straight from SBUF by the
            # combine; the other quarters roundtrip through sh_dram
            oaf = None
            if q < 3:
                def oaf(sc, q=q):
                    return sh_dram[q * 512 + sc * 128:
                                   q * 512 + (sc + 1) * 128, :]
            return _emit_ffn_unit(
                nc, pools,
                rhs_fn=lambda dc, q=q: xTb_q[q][dc][:],
                out_ap_fn=oaf, keys_ap=sk_d, vals_ap=sv_d, **kw)

        # ---------- software-pipelined router + shared FFN ----------
        emit_A(0); emit_B(0); emit_C(0)
        emit_A(1); emit_B(1)
        _, kbn = emit_shared(0, next_keys_ap=sk_d)
        emit_EF(0); emit_C(1)
        emit_A(2); emit_B(2)
        _, kbn = emit_shared(1, kb_pre=kbn, next_keys_ap=sk_d)
        emit_EF(1); emit_C(2)
        emit_A(3); emit_B(3)
        _, kbn = emit_shared(2, kb_pre=kbn, next_keys_ap=k_d[0])
        emit_EF(2); emit_C(3); emit_EF(3)
        es_rt.close()

        # ---------- combine pools (open in freed router space; LIFO: close
        # before the ffn pools) ----------
        p_eg = mk(es_cb, "p_eg", 12)
        p_shg = mk(es_cb, "p_shg", 8)
        p_ot = mk(es_cb, "p_ot", 3)
        cb_state = [None] * NG

        def cb_prefetch(g):
            eg = [p_eg.tile([128, D], BF16, tag="eg", name="eg")
                  for _ in range(2)]
            for ch in range(2):
                nc.sync.dma_start(eg[ch][:],
                                  eo_dram[ch][g * 128:(g + 1) * 128, :])
            if g < 12:
                shg = p_shg.tile([128, D], BF16, tag="shg", name="shg")
                nc.scalar.dma_start(shg[:],
                                    sh_dram[g * 128:(g + 1) * 128, :])
            else:
                shg = None  # groups 12-15 add SH3's eo tiles straight from SBUF
            cb_state[g] = (eg, shg)

        def cb_compute(g, shg_override=None):
            eg, shg = cb_state[g]
            if shg_override is not None:
                shg = shg_override
            psA = ps.tile([128, 512], F32, tag="ps", name="ps")
            psB = ps.tile([128, 512], F32, tag="ps", name="ps")
            for ch in range(2):
                lhsT = combT[ch][:, g * 128:(g + 1) * 128]
                nc.tensor.matmul(psA[:], lhsT, eg[ch][:, 0:512],
                                 start=(ch == 0), stop=(ch == 1))
                nc.tensor.matmul(psB[:], lhsT, eg[ch][:, 512:1024],
                                 start=(ch == 0), stop=(ch == 1))
            ot = p_ot.tile([128, D], BF16, tag="ot", name="ot")
            nc.vector.tensor_tensor(ot[:, 0:512], psA[:], shg[:, 0:512],
                                    op=ALU.add)
            nc.vector.tensor_tensor(ot[:, 512:1024], psB[:],
                                    shg[:, 512:1024], op=ALU.add)
            nc.gpsimd.dma_start(out_d[g * 128:(g + 1) * 128, :], ot[:])

        # ---------- routed expert units ----------
        eo_g = [t[:].rearrange("(g ec) d -> g ec d", ec=128)
                for t in eo_dram]
        for e in range(E):
            def rhs_fn(dc, e=e):
                r = dispT[dc][:].rearrange("p (g ec) -> p g ec", g=NG)
                return r[:, :, e * CAP:(e + 1) * CAP]
            def out_ap_fn(sc, e=e):
                # FFN2 tile rows are slots (g-major): row r -> group
                # sc*4 + r//32, capacity slot r%32 of expert e
                return eo_g[e // 4][sc * 4:(sc + 1) * 4,
                                    (e % 4) * CAP:(e % 4 + 1) * CAP, :]
            nka = k_d[e + 1] if e < E - 1 else sk_d
            _, kbn = _emit_ffn_unit(nc, pools, rhs_fn=rhs_fn,
                                    out_ap_fn=out_ap_fn, keys_ap=k_d[e],
                                    vals_ap=v_d[e], kb_pre=kbn,
                                    next_keys_ap=nka)

        # last shared unit runs after the experts with the combine woven in:
        # each group's expert outputs prefetch a few FFN1 blocks ahead and its
        # 4 combine matmuls fill the PSUM-free windows between FFN1 blocks.
        # No prefetch at weave(0): an eg DMA still waiting on expert 7's eo
        # writes would head-of-line block SH3's keys stream on sync.
        def sh3_weave(b):
            if b == 0:
                return
            if b >= 2:
                cb_compute(b - 2)
            pf = {1: (0, 1), 2: (2, 3), 3: (4, 5), 4: (6, 7),
                  5: (8,), 6: (9,), 7: (10,)}[b]
            for g in pf:
                cb_prefetch(g)

        def sh3_post_ffn1():
            for g in range(6, 10):
                cb_compute(g)
            for g in range(11, 16):
                cb_prefetch(g)

        sh3_eo, _ = emit_shared(3, weave=sh3_weave, post_ffn1=sh3_post_ffn1,
                                kb_pre=kbn)
        for g in range(10, 12):
            cb_compute(g)
        for g in range(12, NG):
            cb_compute(g, shg_override=sh3_eo[g - 12])

        es_cb.close()
        es_ffn.close()
        es_xtb.close()
        es_dT.close()

    nc.compile()
    return nc


LAST_RESULT = None


def kernel(x, gate_weight, gate_bias, keys, values, shared_keys,
           shared_values, **_ignored):
    global LAST_RESULT
    _ensure_ntff_hook()
    BF = ml_dtypes.bfloat16
    x = np.ascontiguousarray(np.asarray(x, dtype=np.float32))
    gate_weight = np.ascontiguousarray(np.asarray(gate_weight, np.float32))
    gate_bias = np.ascontiguousarray(
        np.asarray(gate_bias, np.float32).reshape(1, E))
    keys = np.ascontiguousarray(np.asarray(keys, np.float32).astype(BF))
    values = np.ascontiguousarray(np.asarray(values, np.float32).astype(BF))
    shared_keys = np.ascontiguousarray(
        np.asarray(shared_keys, np.float32).reshape(D, H).astype(BF))
    shared_values = np.ascontiguousarray(
        np.asarray(shared_values, np.float32).reshape(H, D).astype(BF))

    if "nc" not in _CACHE:
        _CACHE["nc"] = _build_program()
    nc = _CACHE["nc"]

    xt = x.reshape(NCORES, TOK, D)
    in_maps = []
    for i in range(NCORES):
        in_maps.append({
            "x_s": np.ascontiguousarray(xt[i]),
            "gw": gate_weight,
            "gb": gate_bias,
            "keys": keys,
            "values": values,
            "shk": shared_keys,
            "shv": shared_values,
        })
    res = run_bass_kernel_spmd(nc, in_maps, core_ids=list(range(NCORES)))
    LAST_RESULT = res
    out = np.concatenate([res.results[i]["out"] for i in range(NCORES)],
                         axis=0)
    return out.reshape(B, S, D).astype(np.float32)
